# revision 24
# baseline (speedup 1.0000x reference)
"""CapsNet forward on 8 Trainium2 NeuronCores (Bass/Tile).

Strategy (v2):
  - conv1 (9x9 s1 + relu) as im2col matmul in fp16, writing x1 in fp8
    (x64 scale) with layout [p, icb, ph, pw, h', w', b].
  - conv2 (9x9 s2) in fp8e4m3 with DoubleRow perf mode: K-pairs over the
    two input-channel blocks; moving operand merges (w-window x batch)
    into one contiguous dim; 4 chunk-major accumulation groups (m, oh-half).
  - primary squash (value-threshold form) + u_sq = mag * u in fp16.
  - single AllToAll (fp16) to route-parallel: dest d owns channels
    {m*128 + d*16 + c : c<16, m in 0,1}; payload [dest][b][r', i] so the
    receive side is one uniform-stride DMA.
  - routing (3 iters): s_j via [(r,i) x b]^T @ (exp(b_ij) . W) fp16
    matmuls, fused AllReduce carrying [s_tilde | sum_exp]; agreement via
    T = u_sq^T v + comb-matmul; digit squash exact rank arithmetic fp32.
  - decoder computed for all 104 rows on every core (identical results);
    L1/L2 weight-stationary (bias fused per-partition), L3 moving-form
    with bias as a K=1 matmul row; core 0's output is used by the host.
"""

import numpy as np
import ml_dtypes

import concourse.bass as bass
import concourse.mybir as mybir
import concourse.tile as tile
from concourse import bacc
from concourse.bass_utils import run_bass_kernel_spmd
from concourse.masks import make_identity
from concourse import bass_isa

F32 = mybir.dt.float32
I32 = mybir.dt.int32
F16 = mybir.dt.float16
F8 = mybir.dt.float8e4
AX = mybir.AxisListType
OP = mybir.AluOpType
ACT = mybir.ActivationFunctionType
DR = mybir.MatmulPerfMode.DoubleRow

NCORES = 8
BL = 13            # batch rows per core
BG = NCORES * BL   # 104 (padded batch)
NR, NC_, DI, DO = 2048, 10, 8, 16
RSH = NR // NCORES  # 256 routes per core
CO = NC_ * DO       # 160
RI = RSH * DI       # 2048 = (r', i) per core
XS = 32.0           # x1 fp8 scale (TRN2 fp8e4 saturates at 240)
WS = 4096.0         # conv2 weight fp8 scale

PRIM = (-13.46416092, 0.000242759, 0.024488359, 0.002769205, 0.06089699,
        13.23405266, -0.002828244, 0.061313814, -0.000219038, 0.023874787)
DIGIT = (-0.075410217, -0.074520095, 0.349297946, -0.534473989, 0.27196494,
         0.062207676, 0.637642944, 0.295330779, 0.169344703, 0.353784456)


def _ap(t, offset, dims):
    return bass.AP(tensor=t, offset=offset, ap=[list(d) for d in dims])


def build_program():
    nc = bacc.Bacc("TRN2", target_bir_lowering=False, debug=False,
                   num_devices=NCORES)

    # ---------------- I/O ----------------
    r1c = nc.dram_tensor("r1c", [81, BL * 576], F16, kind="ExternalInput")
    c1w = nc.dram_tensor("c1w", [81, 256], F16, kind="ExternalInput")
    c1b = nc.dram_tensor("c1b", [128, 2], F32, kind="ExternalInput")
    c2w8 = nc.dram_tensor("c2w8", [128, 2 * 81 * 256], F8,
                          kind="ExternalInput")
    c2b = nc.dram_tensor("c2b", [128, 2], F32, kind="ExternalInput")
    wre = nc.dram_tensor("wre", [RI, CO], F16, kind="ExternalInput")
    comb = nc.dram_tensor("comb", [128, 128], F16, kind="ExternalInput")
    bmask = nc.dram_tensor("bmask", [BG, 1], F32, kind="ExternalInput")
    d1 = nc.dram_tensor("d1", [160, 512], F16, kind="ExternalInput")
    d1b = nc.dram_tensor("d1b", [128, 4], F32, kind="ExternalInput")
    d2 = nc.dram_tensor("d2", [512, 1024], F16, kind="ExternalInput")
    d2br = nc.dram_tensor("d2br", [1, 1024], F16, kind="ExternalInput")
    d3 = nc.dram_tensor("d3", [1024, 1024], F16, kind="ExternalInput")
    d3br = nc.dram_tensor("d3br", [1, 1024], F16, kind="ExternalInput")
    out = nc.dram_tensor("out", [BG, 1184], F32, kind="ExternalOutput")

    # internal DRAM (collective bounce buffers); u_sq ships as fp8 (x8)
    usq_send = nc.dram_tensor("usq_send", [NCORES, BL, RI], F8)
    usq_recv = nc.dram_tensor("usq_recv", [NCORES, BL, RI], F8)
    CCN = BG * CO + 16  # 16656
    ccw_in = nc.dram_tensor("ccw_in", [16], F16)
    ccw_out = nc.dram_tensor("ccw_out", [16], F16, addr_space="Shared")
    cc_in = [nc.dram_tensor(f"cc_in{i}", [CCN], F16) for i in range(3)]
    cc_out = [nc.dram_tensor(f"cc_out{i}", [CCN], F16, addr_space="Shared")
              for i in range(3)]
    GROUPS = [list(range(NCORES))]

    t1, a1, b1, a2, b2, t3, a3, b3, a4, b4 = [float(v) for v in PRIM]
    dt1, da1, db1, da2, db2, dt3, da3, db3, da4, db4 = [float(v) for v in DIGIT]

    with tile.TileContext(nc) as tc:
        const = tc.alloc_tile_pool(name="const", bufs=1)
        z16 = const.tile([1, 16], F16)
        nc.gpsimd.memset(z16[:], 0.0)
        nc.gpsimd.dma_start(_ap(ccw_in[:].tensor, 0, [[16, 1], [1, 16]]),
                            z16[:])
        # warm-up collective: absorbs the first-collective barrier while
        # the conv phase runs
        nc.gpsimd.collective_compute(
            "AllReduce", OP.add, replica_groups=GROUPS,
            ins=[ccw_in[:]], outs=[ccw_out[:]])
        ident = const.tile([128, 128], F32)
        make_identity(nc, ident[:])
        ident16 = const.tile([128, 128], F16)
        nc.vector.tensor_copy(ident16[:], ident[:])
        c1b_sb = const.tile([128, 2], F32)
        nc.gpsimd.dma_start(c1b_sb[:], c1b[:, :])
        c2b_sb = const.tile([128, 2], F32)
        nc.gpsimd.dma_start(c2b_sb[:], c2b[:, :])
        comb_sb = const.tile([128, 128], F16)
        nc.gpsimd.dma_start(comb_sb[:], comb[:, :])
        bmask_sb = const.tile([BG, 1], F32)
        nc.gpsimd.dma_start(bmask_sb[:], bmask[:, :])
        ones8 = const.tile([128, 1], F16)
        nc.gpsimd.memset(ones8[:], 0.125)
        ones104 = const.tile([BG, 1], F32)
        nc.gpsimd.memset(ones104[:], 1.0)
        ones_r104 = const.tile([1, BG], F32)
        nc.gpsimd.memset(ones_r104[:], 1.0)
        ones1_16 = const.tile([1, BG], F16)
        nc.gpsimd.memset(ones1_16[:], 1.0)
        negbig = const.tile([128, 1], F32)
        nc.gpsimd.memset(negbig[:], -1e30)
        # zero the unused tail slots read by the fused collectives
        nc.gpsimd.dma_start(
            _ap(cc_in[0][:].tensor, BG * CO, [[16, 1], [1, 16]]), z16[:])
        for it in (1, 2):
            nc.gpsimd.dma_start(
                _ap(cc_in[it][:].tensor, BG * CO + 10, [[6, 1], [1, 6]]),
                z16[0:1, 0:6])

        persist = tc.alloc_tile_pool(name="persist", bufs=1)
        sq = tc.alloc_tile_pool(name="sq", bufs=1)
        rt = tc.alloc_tile_pool(name="routing", bufs=1)
        W16 = rt.tile([128, 16, CO], F16)
        usq_b = rt.tile([128, RI], F16)  # [b, (r', i)]
        usq_T = rt.tile([128, 16, BG], F16)
        b_rep = rt.tile([128, CO], F32)
        nc.gpsimd.memset(b_rep[:], 0.0)
        vj = rt.tile([BG, CO], F32)  # final v_j lives here after it=2

        # x1 in fp8 (scaled x64): [p, icb, ph, pw, h', w', b]
        x1a = persist.tile([128, 2, 2, 2, 12, 12, BL], F8)
        c2w_sb = persist.tile([128, 2, 81, 256], F8)

        dc = tc.alloc_tile_pool(name="dec", bufs=1)

        # startup DMAs in priority order on the sync queue
        with tc.tile_pool(name="conv1", bufs=1) as c1pool:
            r1 = c1pool.tile([81, BL * 576], F16)
            nc.sync.dma_start(r1[:], r1c[:, :])
            c1w_sb = c1pool.tile([81, 256], F16)
            nc.sync.dma_start(c1w_sb[:], c1w[:, :])
            # split by oc half so conv2's m=0 groups can start sooner
            for m in range(2):
                for icb in range(2):
                    nc.sync.dma_start(
                        c2w_sb[:, icb, :, m * 128:(m + 1) * 128],
                        _ap(c2w8[:, :].tensor, icb * 81 * 256 + m * 128,
                            [[2 * 81 * 256, 128], [256, 81], [1, 128]]))
            nc.sync.dma_start(
                W16[:], _ap(wre[:, :].tensor, 0,
                            [[CO, 128], [128 * CO, 16], [1, CO]]))

            # decoder weights (stream under the conv phase)
            dwsb = {}
            for nm, (kdim, ndim, win_dram) in (
                    ("1", (160, 512, d1)),
                    ("2", (512, 1024, d2)),
                    ("3", (1024, 1024, d3))):
                nkt = (kdim + 127) // 128
                wsb = dc.tile([128, nkt, ndim], F16, tag=f"w{nm}",
                              name=f"w{nm}")
                for kt in range(nkt):
                    ksz = min(128, kdim - kt * 128)
                    nc.sync.dma_start(
                        wsb[:ksz, kt, :],
                        _ap(win_dram[:, :].tensor, kt * 128 * ndim,
                            [[ndim, ksz], [1, ndim]]))
                dwsb[nm] = wsb
            d1b_sb = dc.tile([128, 4], F32)
            nc.sync.dma_start(d1b_sb[:], d1b[:, :])
            d2br_sb = dc.tile([1, 1024], F16)
            nc.sync.dma_start(d2br_sb[:], d2br[:, :])
            d3br_sb = dc.tile([1, 1024], F16)
            nc.sync.dma_start(d3br_sb[:], d3br[:, :])

            # ====== conv1: r1c -> x1 fp8 [icb, ph, pw, h', w', b] ======
            with tc.tile_pool(name="c1psum", bufs=2, space="PSUM") as c1ps:
                NTOT = BL * 576  # 7488 per m
                for m in range(2):
                    off = 0
                    while off < NTOT:
                        csz = min(512, NTOT - off)
                        ps = c1ps.tile([128, 512], F32, tag="c1ps")
                        nc.tensor.matmul(ps[:, :csz],
                                         c1w_sb[0:81, m * 128:(m + 1) * 128],
                                         r1[0:81, off:off + csz])
                        xh = x1a[:, m].rearrange(
                            "p a c h w b -> p (a c h w b)")[:, off:off + csz]
                        nc.scalar.activation(xh, ps[:, :csz],
                                             ACT.Relu, bias=c1b_sb[:, m:m + 1])
                        off += csz

        # ============ conv2: fp8 DoubleRow, 4 chunk groups ============
        u_t = [persist.tile([128, BL, 8, 8], F32, tag=f"u_{m}",
                            name=f"u_{m}") for m in range(2)]
        hmax = sq.tile([128, 2, BL], F32)    # [c, m, b]
        hneg = sq.tile([128, 2, BL], F32)
        with tc.tile_pool(name="c2psum", bufs=2, space="PSUM") as c2ps:
            for m in range(2):
                for hc in range(2):
                    ps = c2ps.tile([128, 4, 104], F32, tag="c2ps")
                    for j in range(81):
                        kh, kw = divmod(j, 9)
                        ph, h0 = kh & 1, kh >> 1
                        pw, w0 = kw & 1, kw >> 1
                        rhs = _ap(x1a[:].tensor,
                                  x1a[:].offset + ph * 3744 + pw * 1872
                                  + (hc * 4 + h0) * 156 + w0 * 13,
                                  [list(x1a[:].ap[0]), [7488, 2],
                                   [156, 4], [1, 104]])
                        nc.tensor.matmul(ps[:], c2w_sb[:, :, j,
                                                       m * 128:(m + 1) * 128],
                                         rhs, start=(j == 0), stop=(j == 80),
                                         perf_mode=DR)
                    # readout: psum [p, oh(4), w(8), b(13)] -> u_t [p,b,oh,w]
                    pst = ps[:]
                    src = _ap(pst.tensor, pst.offset,
                              [list(pst.ap[0]), [1, BL], [104, 4], [13, 8]])
                    ut = u_t[m][:]
                    dst = _ap(ut.tensor, ut.offset + hc * 4 * 8,
                              [list(ut.ap[0]), [64, BL], [8, 4], [1, 8]])
                    nc.scalar.activation(dst, src, ACT.Identity,
                                         bias=c2b_sb[:, m:m + 1],
                                         scale=1.0 / (XS * WS))
                    # incremental squash maxima (hidden under next chunk)
                    xs = u_t[m][:, :, hc * 4:hc * 4 + 4, 0]  # [128, b, 4h]
                    red = sq.tile([128, BL], F32, tag="red")
                    nc.vector.tensor_reduce(red[:], xs, AX.X, OP.max)
                    if hc == 0:
                        nc.vector.tensor_copy(hmax[:, m, :], red[:])
                    else:
                        nc.vector.tensor_tensor(hmax[:, m, :], hmax[:, m, :],
                                                red[:], OP.max)
                    msk = sq.tile([128, BL, 4], I32, tag="msk")
                    nc.vector.tensor_single_scalar(msk[:], xs, 0.0, OP.is_lt)
                    xn = sq.tile([128, BL, 4], F32, tag="xn")
                    nc.vector.tensor_copy(
                        xn[:], negbig[:, 0:1].to_broadcast((128, BL, 4)))
                    nc.vector.copy_predicated(xn[:], msk[:], xs)
                    nc.vector.tensor_reduce(red[:], xn[:], AX.X, OP.max)
                    if hc == 0:
                        nc.vector.tensor_copy(hneg[:, m, :], red[:])
                    else:
                        nc.vector.tensor_tensor(hneg[:, m, :], hneg[:, m, :],
                                                red[:], OP.max)

        # ======== primary squash (value-threshold form) + u_sq ========
        # cross-partition max, replicated to all partitions
        redM = sq.tile([128, 2 * BL], F32)
        redN = sq.tile([128, 2 * BL], F32)
        hmax2 = hmax[:].rearrange("p m b -> p (m b)")
        hneg2 = hneg[:].rearrange("p m b -> p (m b)")
        nc.gpsimd.partition_all_reduce(redM[:], hmax2, channels=128,
                                       reduce_op=bass_isa.ReduceOp.max)
        nc.gpsimd.partition_all_reduce(redN[:], hneg2, channels=128,
                                       reduce_op=bass_isa.ReduceOp.max)
        Mb = sq.tile([128, BL], F32)
        Nb = sq.tile([128, BL], F32)
        nc.vector.tensor_tensor(Mb[:], redM[:, 0:BL],
                                redM[:, BL:2 * BL], OP.max)
        nc.vector.tensor_tensor(Nb[:], redN[:, 0:BL],
                                redN[:, BL:2 * BL], OP.max)

        usq = [persist.tile([128, BL, 8, 8], F8, tag=f"usq_{m}",
                            name=f"usq_{m}") for m in range(2)]
        for m in range(2):
            xs = u_t[m][:, :, :, 0]          # [128, b, h]
            y = sq.tile([128, BL, 8], F32, tag="y")
            aff = sq.tile([128, BL, 8], F32, tag="aff")
            mk = sq.tile([128, BL, 8], I32, tag="mk")
            mk2 = sq.tile([128, BL, 8], I32, tag="mk2")
            nc.vector.tensor_copy(y[:], xs)
            # x < mneg -> a2*x+b2
            nc.vector.tensor_tensor(
                mk[:], xs, Nb[:, :, None].to_broadcast((128, BL, 8)),
                OP.is_lt)
            nc.vector.tensor_scalar(aff[:], xs, a2, b2, OP.mult, OP.add)
            nc.vector.copy_predicated(y[:], mk[:], aff[:])
            # (x >= 0) & (x < M) -> a3*x+b3
            nc.vector.tensor_single_scalar(mk[:], xs, 0.0, OP.is_ge)
            nc.vector.tensor_tensor(
                mk2[:], xs, Mb[:, :, None].to_broadcast((128, BL, 8)),
                OP.is_lt)
            nc.vector.tensor_tensor(mk[:], mk[:], mk2[:], OP.mult)
            nc.vector.tensor_scalar(aff[:], xs, a3, b3, OP.mult, OP.add)
            nc.vector.copy_predicated(y[:], mk[:], aff[:])
            # u_sq_fp8 = 8 * y * u  (x8 for fp8 range; undone on receive)
            nc.vector.tensor_scalar(y[:], y[:], 8.0, None, OP.mult)
            nc.vector.tensor_tensor(
                usq[m][:], u_t[m][:],
                y[:, :, :, None].to_broadcast((128, BL, 8, 8)), OP.mult)

        # scatter to send buffer [dest][b][(ch'=m*16+c)*64 + oh*8 + i]
        engs = [nc.sync, nc.scalar, nc.gpsimd]
        for m in range(2):
            for d in range(NCORES):
                dst = _ap(usq_send[:].tensor, d * (BL * RI) + m * 1024,
                          [[64, 16], [RI, BL], [1, 64]])
                engs[(m * NCORES + d) % 3].dma_start(
                    dst, usq[m][d * 16:(d + 1) * 16, :, :, :])

        # ============ AllToAll: u_sq -> route-sharded, full batch ========
        nc.gpsimd.collective_compute(
            "AllToAll", OP.bypass, replica_groups=GROUPS,
            ins=[usq_send[:]], outs=[usq_recv[:]])

        # ============ routing ============
        usq8_b = rt.tile([128, RI], F8)
        nc.sync.dma_start(
            usq8_b[0:BG, :], _ap(usq_recv[:].tensor, 0, [[RI, BG], [1, RI]]))
        with tc.tile_pool(name="tps", bufs=2, space="PSUM") as tps:
            # p-state warmers: keep the PE clocked up through the AllToAll
            psW = tps.tile([128, BG], F32, tag="psW")
            for dk in range(40):
                nc.tensor.matmul(psW[:], ident16[:, 0:128],
                                 ident16[:, 0:BG], start=True, stop=True)
            # un-scale the fp8 payload back to fp16 u_sq
            nc.vector.tensor_scalar(usq_b[0:BG, :], usq8_b[0:BG, :],
                                    0.125, None, OP.mult)
            for t in range(16):
                pt = tps.tile([128, BG], F16, tag="pt")
                nc.tensor.transpose(pt[:], usq_b[0:BG, 128 * t:128 * (t + 1)],
                                    ident16[0:BG, 0:BG])
                nc.vector.tensor_copy(usq_T[:, t, :], pt[:])

        with tc.tile_pool(name="rloop", bufs=3) as rl, \
             tc.tile_pool(name="rpsS", bufs=1, space="PSUM") as rpsS, \
             tc.tile_pool(name="rpsT", bufs=1, space="PSUM") as rpsT, \
             tc.tile_pool(name="rps1", bufs=1, space="PSUM") as rps1:
            for it in range(3):
                if it == 0:
                    # b_ij = 0: c = W, E_c = 2048 exactly
                    mc = W16
                else:
                    cexp = rl.tile([128, CO], F16, tag="cexp")
                    nc.scalar.activation(cexp[:], b_rep[:], ACT.Exp)
                    mc = rl.tile([128, 16, CO], F16, tag="mc")
                    cexp_b = _ap(cexp[:].tensor, cexp[:].offset,
                                 [list(cexp[:].ap[0]), [10, 16], [1, 10],
                                  [0, 16]])
                    nc.vector.tensor_tensor(
                        mc[:].rearrange("p t (c o) -> p t c o", c=10),
                        W16[:].rearrange("p t (c o) -> p t c o", c=10),
                        cexp_b, OP.mult)
                    # E_c partial
                    psE = rps1.tile([1, CO], F32, tag="psE")
                    nc.tensor.matmul(psE[:], ones8[:], cexp[:])
                    E10 = rl.tile([1, 10], F16, tag="E10")
                    psE_v = _ap(psE[:].tensor, psE[:].offset,
                                [list(psE[:].ap[0]), [1, 10], [10, 16]])
                    with nc.allow_low_precision(
                            reason="E sums ~2048, fp16 ok (validated)"):
                        nc.vector.tensor_reduce(E10[:], psE_v, AX.X, OP.add)
                # s_tilde
                psS = rpsS.tile([BG, CO], F32, tag="psS")
                for t in range(16):
                    nc.tensor.matmul(psS[:], usq_T[:, t, :], mc[:, t, :],
                                     start=(t == 0), stop=(t == 15))
                s_sb = rl.tile([BG, CO], F16, tag="s_sb")
                nc.vector.tensor_copy(s_sb[:], psS[:])
                nc.sync.dma_start(
                    _ap(cc_in[it][:].tensor, 0, [[CO, BG], [1, CO]]), s_sb[:])
                if it > 0:
                    nc.sync.dma_start(
                        _ap(cc_in[it][:].tensor, BG * CO, [[1, 1], [1, 10]]),
                        E10[:])
                # p-state warmers: keep the PE clocked up through the
                # collective so post-AR matmuls run at full rate
                for dk in range(36):
                    nc.tensor.matmul(psS[:, 0:BG], usq_T[:, 0, :],
                                     usq_T[:, 1, :], start=True, stop=True)
                nc.gpsimd.collective_compute(
                    "AllReduce", OP.add, replica_groups=GROUPS,
                    ins=[cc_in[it][:]], outs=[cc_out[it][:]])
                s_full = rl.tile([BG, CO], F16, tag="s_full")
                nc.sync.dma_start(
                    s_full[:],
                    _ap(cc_out[it][:].tensor, 0, [[CO, BG], [1, CO]]))
                sj = rl.tile([BG, CO], F32, tag="sj")
                if it == 0:
                    nc.vector.tensor_scalar(sj[:], s_full[:], 1.0 / 2048.0,
                                            None, OP.mult)
                else:
                    E10r = rl.tile([1, 10], F16, tag="E10r")
                    nc.sync.dma_start(
                        E10r[:],
                        _ap(cc_out[it][:].tensor, BG * CO, [[1, 1], [1, 10]]))
                    E32 = rl.tile([1, 10], F32, tag="E32")
                    nc.vector.tensor_copy(E32[:], E10r[:])
                    rE = rl.tile([1, 10], F32, tag="rE")
                    nc.vector.reciprocal(rE[:], E32[:])
                    psBE = rps1.tile([BG, CO], F32, tag="psBE")
                    rE_b = _ap(rE[:].tensor, rE[:].offset,
                               [list(rE[:].ap[0]), [1, 10], [0, 16]])
                    nc.tensor.matmul(psBE[:], ones_r104[:], rE_b)
                    nc.vector.tensor_tensor(sj[:], s_full[:], psBE[:],
                                            OP.mult)

                # ---- digit squash (exact rank arithmetic) ----
                x10 = _ap(sj[:].tensor, sj[:].offset,
                          [list(sj[:].ap[0]), [16, 10]])
                cmp = rl.tile([BG, 10, 10], F32, tag="cmp")
                x_j = _ap(sj[:].tensor, sj[:].offset,
                          [list(sj[:].ap[0]), [16, 10], [0, 10]])
                x_k = _ap(sj[:].tensor, sj[:].offset,
                          [list(sj[:].ap[0]), [0, 10], [16, 10]])
                nc.vector.tensor_tensor(cmp[:], x_j, x_k, OP.is_gt)
                r10 = rl.tile([BG, 10], F32, tag="r10")
                nc.vector.tensor_reduce(r10[:], cmp[:], AX.X, OP.add)
                y = rl.tile([BG, 10], F32, tag="y")
                tmp = rl.tile([BG, 10], F32, tag="tmp")
                aff = rl.tile([BG, 10], F32, tag="aff")
                mkA = rl.tile([BG, 10], I32, tag="mkA")
                mkB = rl.tile([BG, 10], I32, tag="mkB")
                cnt = rl.tile([BG, 4], F32, tag="cnt")  # i1, i2, i3 columns
                # i1
                nc.vector.tensor_single_scalar(tmp[:], x10, dt1, OP.is_lt)
                nc.vector.tensor_reduce(cnt[:, 0:1], tmp[:], AX.X, OP.add)
                # stage 1: r < i1 - 1
                nc.vector.tensor_copy(y[:], x10)
                nc.vector.tensor_scalar(tmp[:], cnt[:, 0:1].to_broadcast(
                    (BG, 10)), 1.0, None, OP.subtract)
                nc.vector.tensor_tensor(mkA[:], r10[:], tmp[:], OP.is_lt)
                nc.vector.tensor_scalar(aff[:], x10, da1, db1, OP.mult, OP.add)
                nc.vector.copy_predicated(y[:], mkA[:], aff[:])
                # i2 on modified y
                nc.vector.tensor_single_scalar(tmp[:], y[:], 0.0, OP.is_lt)
                nc.vector.tensor_reduce(cnt[:, 1:2], tmp[:], AX.X, OP.add)
                # stage 2: (r >= i1) & (r < i2 - 1)
                nc.vector.tensor_tensor(
                    mkA[:], r10[:], cnt[:, 0:1].to_broadcast((BG, 10)),
                    OP.is_ge)
                nc.vector.tensor_scalar(tmp[:], cnt[:, 1:2].to_broadcast(
                    (BG, 10)), 1.0, None, OP.subtract)
                nc.vector.tensor_tensor(mkB[:], r10[:], tmp[:], OP.is_lt)
                nc.vector.tensor_tensor(mkA[:], mkA[:], mkB[:], OP.mult)
                nc.vector.tensor_scalar(aff[:], y[:], da2, db2, OP.mult, OP.add)
                nc.vector.copy_predicated(y[:], mkA[:], aff[:])
                # i3 on modified y
                nc.vector.tensor_single_scalar(tmp[:], y[:], dt3, OP.is_lt)
                nc.vector.tensor_reduce(cnt[:, 2:3], tmp[:], AX.X, OP.add)
                # stage 3: (r >= i2) & (r < i3 - 1)
                nc.vector.tensor_tensor(
                    mkA[:], r10[:], cnt[:, 1:2].to_broadcast((BG, 10)),
                    OP.is_ge)
                nc.vector.tensor_scalar(tmp[:], cnt[:, 2:3].to_broadcast(
                    (BG, 10)), 1.0, None, OP.subtract)
                nc.vector.tensor_tensor(mkB[:], r10[:], tmp[:], OP.is_lt)
                nc.vector.tensor_tensor(mkA[:], mkA[:], mkB[:], OP.mult)
                nc.vector.tensor_scalar(aff[:], y[:], da3, db3, OP.mult, OP.add)
                nc.vector.copy_predicated(y[:], mkA[:], aff[:])
                # stage 4: (r >= i3) & (r < 9)
                nc.vector.tensor_tensor(
                    mkA[:], r10[:], cnt[:, 2:3].to_broadcast((BG, 10)),
                    OP.is_ge)
                nc.vector.tensor_single_scalar(mkB[:], r10[:], 9.0, OP.is_lt)
                nc.vector.tensor_tensor(mkA[:], mkA[:], mkB[:], OP.mult)
                nc.vector.tensor_scalar(aff[:], y[:], da4, db4, OP.mult, OP.add)
                nc.vector.copy_predicated(y[:], mkA[:], aff[:])
                # v_j = f * s_mod (s_mod[:, :, 0] = f)
                if it == 2:
                    vdst = vj
                else:
                    vdst = rl.tile([BG, CO], F32, tag="vtmp", name="vtmp")
                nc.vector.tensor_copy(vdst[:], sj[:])
                vdst0 = _ap(vdst[:].tensor, vdst[:].offset,
                            [list(vdst[:].ap[0]), [16, 10]])
                nc.vector.tensor_copy(vdst0, y[:])
                f_b = _ap(y[:].tensor, y[:].offset,
                          [list(y[:].ap[0]), [1, 10], [0, 16]])
                nc.vector.tensor_tensor(
                    vdst[:].rearrange("b (c o) -> b c o", c=10),
                    vdst[:].rearrange("b (c o) -> b c o", c=10), f_b, OP.mult)

                if it < 2:
                    v16 = rl.tile([BG, CO], F16, tag="v16", name="v16")
                    nc.vector.tensor_copy(v16[:], vdst[:])
                    qall = rl.tile([128, 16, 10], F16, tag="qall")
                    for half in range(2):
                        psT = rpsT.tile([128, 8, 256], F32, tag="psT")
                        for j in range(8):
                            t = half * 8 + j
                            nc.tensor.matmul(
                                psT[:, j, 0:CO],
                                usq_b[0:BG, 128 * t:128 * (t + 1)], v16[:])
                        prod = rl.tile([128, 8, CO], F16, tag="prod")
                        nc.vector.tensor_tensor(
                            prod[:], W16[:, 8 * half:8 * (half + 1), :],
                            psT[:, :, 0:CO], OP.mult)
                        with nc.allow_low_precision(
                                reason="16-term o-sum feeding small logits"):
                            nc.vector.tensor_reduce(
                                qall[:, 8 * half:8 * (half + 1), :],
                                prod[:].rearrange("p j (c o) -> p j c o",
                                                  c=10),
                                AX.X, OP.add)
                    psA = rpsS.tile([128, CO], F32, tag="psA")
                    nc.tensor.matmul(psA[:], comb_sb[:],
                                     qall[:].rearrange("p t c -> p (t c)"))
                    nc.vector.tensor_tensor(b_rep[:], b_rep[:], psA[:], OP.add)

        # ============ decoder (all 104 rows, identical on every core) ====
        with tc.tile_pool(name="dps", bufs=2, space="PSUM") as dps:
            sqv = dc.tile([BG, CO], F32)
            nc.scalar.activation(sqv[:], vj[:], ACT.Square)
            csum = dc.tile([BG, 10], F32)
            sq_v = _ap(sqv[:].tensor, sqv[:].offset,
                       [list(sqv[:].ap[0]), [16, 10], [1, 16]])
            nc.vector.tensor_reduce(csum[:], sq_v, AX.X, OP.add)
            classes = dc.tile([BG, 10], F32)
            nc.scalar.activation(classes[:], csum[:], ACT.Sqrt)
            expcl = dc.tile([BG, 10], F32)
            nc.scalar.activation(expcl[:], classes[:], ACT.Exp)
            nc.vector.tensor_scalar_mul(expcl[:], expcl[:], bmask_sb[:, 0:1])
            psD = dps.tile([10, 1], F32, tag="dsmall")
            nc.tensor.matmul(psD[:], expcl[:], ones104[:])
            dsb = dc.tile([10, 1], F32)
            nc.vector.tensor_copy(dsb[:], psD[:])
            psDT = dps.tile([1, 10], F32, tag="dsmall")
            nc.tensor.transpose(psDT[:], dsb[:], ident[0:10, 0:10])
            dT = dc.tile([1, 10], F32)
            nc.vector.tensor_copy(dT[:], psDT[:])
            rD = dc.tile([1, 10], F32)
            nc.vector.reciprocal(rD[:], dT[:])
            psBD = dps.tile([BG, 10], F32, tag="dsmall")
            rD_b = _ap(rD[:].tensor, rD[:].offset,
                       [list(rD[:].ap[0]), [1, 10]])
            nc.tensor.matmul(psBD[:], ones_r104[:], rD_b)
            p = dc.tile([BG, 10], F32)
            nc.vector.tensor_tensor(p[:], expcl[:], psBD[:], OP.mult)
            pm = dc.tile([BG, 1], F32)
            nc.vector.tensor_reduce(pm[:], p[:], AX.X, OP.max)
            mask = dc.tile([BG, 10], F32)
            nc.vector.tensor_tensor(mask[:], p[:],
                                    pm[:].to_broadcast((BG, 10)), OP.is_ge)
            tm = dc.tile([BG, CO], F16)
            mask_b = _ap(mask[:].tensor, mask[:].offset,
                         [list(mask[:].ap[0]), [1, 10], [0, 16]])
            nc.vector.tensor_tensor(
                tm[:].rearrange("b (c o) -> b c o", c=10),
                vj[:].rearrange("b (c o) -> b c o", c=10), mask_b, OP.mult)
            nc.sync.dma_start(out[:, 0:160], vj[:])

            # tT [160, 104] via PE transposes
            tT = dc.tile([128, 2, BG], F16)
            for kt in range(2):
                ksz = 128 if kt == 0 else 32
                pst = dps.tile([128, BG], F16, tag="dpst")
                nc.tensor.transpose(pst[:ksz, :],
                                    tm[:, kt * 128:kt * 128 + ksz],
                                    ident16[0:BG, 0:BG])
                nc.vector.tensor_copy(tT[:ksz, kt, :], pst[:ksz, :])

            # L1/L2 weight-stationary -> transposed activations
            h1T = dc.tile([128, 4, BG], F16)
            for mt in range(4):
                psH = dps.tile([128, BG], F32, tag="dpsH")
                for kt in range(2):
                    ksz = 128 if kt == 0 else 32
                    nc.tensor.matmul(
                        psH[:], dwsb["1"][:ksz, kt, mt * 128:(mt + 1) * 128],
                        tT[:ksz, kt, :], start=(kt == 0), stop=(kt == 1))
                nc.scalar.activation(h1T[:, mt, :], psH[:], ACT.Relu,
                                     bias=d1b_sb[:, mt:mt + 1])
            # L2 moving-form: h2 [104, 1024], bias via K=1 row matmul
            h2 = dc.tile([BG, 1024], F16)
            for half in range(2):
                psH2 = dps.tile([BG, 512], F32, tag="dpsR")
                for kt in range(4):
                    nc.tensor.matmul(
                        psH2[:], h1T[:, kt, :],
                        dwsb["2"][:, kt, half * 512:(half + 1) * 512],
                        start=(kt == 0), stop=False)
                nc.tensor.matmul(
                    psH2[:], ones1_16[:],
                    d2br_sb[0:1, half * 512:(half + 1) * 512],
                    start=False, stop=True)
                nc.scalar.activation(h2[:, half * 512:(half + 1) * 512],
                                     psH2[:], ACT.Relu)
            h2T = dc.tile([128, 8, BG], F16)
            for mt in range(8):
                psh = dps.tile([128, BG], F16, tag="dpst")
                nc.tensor.transpose(psh[:], h2[:, mt * 128:(mt + 1) * 128],
                                    ident16[0:BG, 0:BG])
                nc.vector.tensor_copy(h2T[:, mt, :], psh[:])
            # L3 moving-form: r3 [104, 1024] batch-major, bias via K=1 row
            r3 = dc.tile([BG, 1024], F32)
            for half in range(2):
                psR = dps.tile([BG, 512], F32, tag="dpsR")
                for kt in range(8):
                    nc.tensor.matmul(
                        psR[:], h2T[:, kt, :],
                        dwsb["3"][:, kt, half * 512:(half + 1) * 512],
                        start=(kt == 0), stop=False)
                nc.tensor.matmul(
                    psR[:], ones1_16[:],
                    d3br_sb[0:1, half * 512:(half + 1) * 512],
                    start=False, stop=True)
                nc.scalar.activation(r3[:, half * 512:(half + 1) * 512],
                                     psR[:], ACT.Sigmoid)
            nc.sync.dma_start(out[:, 160:1184], r3[:])

        dc.release()
        rt.release()
        sq.release()
        persist.release()
        const.release()

    nc.compile()
    return nc


_PROGRAM = None


def _get_program():
    global _PROGRAM
    if _PROGRAM is None:
        _PROGRAM = build_program()
    return _PROGRAM


def _prepare_in_maps(inputs):
    data = np.asarray(inputs["data"], dtype=np.float32)      # (100,1,32,32)
    conv1_w = np.asarray(inputs["conv1_w"], dtype=np.float32)
    conv1_b = np.asarray(inputs["conv1_b"], dtype=np.float32)
    prim_w = np.asarray(inputs["prim_w"], dtype=np.float32)
    prim_b = np.asarray(inputs["prim_b"], dtype=np.float32)
    W_dc = np.asarray(inputs["W_dc"], dtype=np.float32)
    dec_w1 = np.asarray(inputs["dec_w1"], dtype=np.float32)
    dec_b1 = np.asarray(inputs["dec_b1"], dtype=np.float32)
    dec_w2 = np.asarray(inputs["dec_w2"], dtype=np.float32)
    dec_b2 = np.asarray(inputs["dec_b2"], dtype=np.float32)
    dec_w3 = np.asarray(inputs["dec_w3"], dtype=np.float32)
    dec_b3 = np.asarray(inputs["dec_b3"], dtype=np.float32)

    B = data.shape[0]
    data_pad = np.zeros((BG, 32, 32), np.float32)
    data_pad[:B] = data[:, 0]
    swv = np.lib.stride_tricks.sliding_window_view(data_pad, (24, 24),
                                                   axis=(1, 2))
    # swv[b, kh, kw, oh, ow] = data[b, oh+kh, ow+kw]
    # columns (ph=oh&1, pw=ow&1, h'=oh>>1, w'=ow>>1, b)
    t5 = swv.transpose(1, 2, 0, 3, 4).reshape(81, BG, 12, 2, 12, 2)
    r1c_all = np.ascontiguousarray(
        t5.transpose(0, 3, 5, 2, 4, 1)).astype(np.float16)  # [81,ph,pw,h,w,b]

    c1w = np.ascontiguousarray(
        conv1_w.transpose(2, 3, 1, 0).reshape(81, 256)).astype(np.float16)
    c1w = (c1w.astype(np.float32) * XS).astype(np.float16)
    c1b = np.zeros((128, 2), np.float32)
    c1b[:, 0] = conv1_b[:128] * XS
    c1b[:, 1] = conv1_b[128:] * XS
    # conv2 weights: [p, icb, tap, oc] * WS -> fp8
    c2w8 = np.ascontiguousarray(
        prim_w.transpose(1, 2, 3, 0).reshape(2, 128, 81, 256)
        .transpose(1, 0, 2, 3)).reshape(128, 2 * 81 * 256)
    c2w8 = (c2w8 * WS).astype(ml_dtypes.float8_e4m3fn)
    c2b = np.zeros((128, 2), np.float32)
    c2b[:, 0] = prim_b[:128]
    c2b[:, 1] = prim_b[128:]
    comb = np.zeros((128, 128), np.float16)
    for blk in range(16):
        comb[blk * 8:(blk + 1) * 8, blk * 8:(blk + 1) * 8] = 0.01
    d1 = np.ascontiguousarray(dec_w1.T).astype(np.float16)
    d1b = np.ascontiguousarray(dec_b1.reshape(4, 128).T)
    d2 = np.ascontiguousarray(dec_w2.T).astype(np.float16)
    d2br = np.ascontiguousarray(dec_b2.reshape(1, 1024)).astype(np.float16)
    d3 = np.ascontiguousarray(dec_w3.T).astype(np.float16)
    d3br = np.ascontiguousarray(dec_b3.reshape(1, 1024)).astype(np.float16)
    bm = np.zeros((BG, 1), np.float32)
    bm[:B] = 1.0

    # route shard: core k, r' = (ch', oh), ch' = m*16 + c_loc,
    # global ch = m*128 + k*16 + c_loc
    rp = np.arange(256)
    chp = rp >> 3
    oh = rp & 7
    m_ = chp >> 4
    c_loc = chp & 15

    in_maps = []
    for k in range(NCORES):
        gch = m_ * 128 + k * 16 + c_loc
        gr = gch * 8 + oh                       # global route index
        # wre rows (r', i): [256, 8, 160] from W_dc[gr] [10, 16, 8]
        wk = W_dc[gr]                           # [256, 10, 16, 8]
        wre = np.ascontiguousarray(
            wk.transpose(0, 3, 1, 2).reshape(RI, CO)).astype(np.float16)
        in_maps.append({
            "r1c": np.ascontiguousarray(
                r1c_all[:, :, :, :, :, k * BL:(k + 1) * BL]
                .reshape(81, BL * 576)),
            "c1w": c1w, "c1b": c1b, "c2w8": c2w8, "c2b": c2b,
            "wre": wre, "comb": comb, "bmask": bm,
            "d1": d1, "d1b": d1b, "d2": d2, "d2br": d2br,
            "d3": d3, "d3br": d3br,
        })

    return in_maps, B


def kernel(**inputs):
    in_maps, B = _prepare_in_maps(inputs)
    nc = _get_program()
    res = run_bass_kernel_spmd(nc, in_maps, list(range(NCORES)))
    return res.results[0]["out"][:B]


def timed_run(inputs):
    in_maps, _ = _prepare_in_maps(inputs)
    nc = _get_program()
    res = run_bass_kernel_spmd(nc, in_maps, list(range(NCORES)), trace=True)
    if res.exec_time_ns is None:
        raise RuntimeError("exec_time_ns unavailable")
    return res.exec_time_ns


# revision 29
# speedup vs baseline: 1.0721x; 1.0721x over previous
"""CapsNet forward on 8 Trainium2 NeuronCores (Bass/Tile).

Strategy (v2):
  - conv1 (9x9 s1 + relu) as im2col matmul in fp16, writing x1 in fp8
    (x64 scale) with layout [p, icb, ph, pw, h', w', b].
  - conv2 (9x9 s2) in fp8e4m3 with DoubleRow perf mode: K-pairs over the
    two input-channel blocks; moving operand merges (w-window x batch)
    into one contiguous dim; 4 chunk-major accumulation groups (m, oh-half).
  - primary squash (value-threshold form) + u_sq = mag * u in fp16.
  - single AllToAll (fp16) to route-parallel: dest d owns channels
    {m*128 + d*16 + c : c<16, m in 0,1}; payload [dest][b][r', i] so the
    receive side is one uniform-stride DMA.
  - routing (3 iters): s_j via [(r,i) x b]^T @ (exp(b_ij) . W) fp16
    matmuls, fused AllReduce carrying [s_tilde | sum_exp]; agreement via
    T = u_sq^T v + comb-matmul; digit squash exact rank arithmetic fp32.
  - decoder computed for all 104 rows on every core (identical results);
    L1/L2 weight-stationary (bias fused per-partition), L3 moving-form
    with bias as a K=1 matmul row; core 0's output is used by the host.
"""

import numpy as np
import ml_dtypes

import concourse.bass as bass
import concourse.mybir as mybir
import concourse.tile as tile
from concourse import bacc
from concourse.bass_utils import run_bass_kernel_spmd
from concourse.masks import make_identity
from concourse import bass_isa

F32 = mybir.dt.float32
I32 = mybir.dt.int32
F16 = mybir.dt.float16
F8 = mybir.dt.float8e4
AX = mybir.AxisListType
OP = mybir.AluOpType
ACT = mybir.ActivationFunctionType
DR = mybir.MatmulPerfMode.DoubleRow

NCORES = 8
BL = 13            # batch rows per core
BG = NCORES * BL   # 104 (padded batch)
NR, NC_, DI, DO = 2048, 10, 8, 16
RSH = NR // NCORES  # 256 routes per core
CO = NC_ * DO       # 160
RI = RSH * DI       # 2048 = (r', i) per core
XS = 32.0           # x1 fp8 scale (TRN2 fp8e4 saturates at 240)
WS = 4096.0         # conv2 weight fp8 scale

PRIM = (-13.46416092, 0.000242759, 0.024488359, 0.002769205, 0.06089699,
        13.23405266, -0.002828244, 0.061313814, -0.000219038, 0.023874787)
DIGIT = (-0.075410217, -0.074520095, 0.349297946, -0.534473989, 0.27196494,
         0.062207676, 0.637642944, 0.295330779, 0.169344703, 0.353784456)


def _ap(t, offset, dims):
    return bass.AP(tensor=t, offset=offset, ap=[list(d) for d in dims])


def build_program():
    nc = bacc.Bacc("TRN2", target_bir_lowering=False, debug=False,
                   num_devices=NCORES)

    # ---------------- I/O ----------------
    r1c = nc.dram_tensor("r1c", [81, BL * 576], F16, kind="ExternalInput")
    c1w = nc.dram_tensor("c1w", [81, 256], F16, kind="ExternalInput")
    c1b = nc.dram_tensor("c1b", [128, 2], F32, kind="ExternalInput")
    c2w8 = nc.dram_tensor("c2w8", [128, 2 * 81 * 256], F8,
                          kind="ExternalInput")
    c2b = nc.dram_tensor("c2b", [128, 2], F32, kind="ExternalInput")
    wre = nc.dram_tensor("wre", [RI, CO], F16, kind="ExternalInput")
    comb = nc.dram_tensor("comb", [128, 128], F16, kind="ExternalInput")
    bmask = nc.dram_tensor("bmask", [BG, 1], F32, kind="ExternalInput")
    d1 = nc.dram_tensor("d1", [160, 512], F16, kind="ExternalInput")
    d1b = nc.dram_tensor("d1b", [128, 4], F32, kind="ExternalInput")
    d2 = nc.dram_tensor("d2", [512, 1024], F16, kind="ExternalInput")
    d2b = nc.dram_tensor("d2b", [128, 8], F32, kind="ExternalInput")
    d3 = nc.dram_tensor("d3", [1024, 1024], F16, kind="ExternalInput")
    d3br = nc.dram_tensor("d3br", [1, 1024], F16, kind="ExternalInput")
    out = nc.dram_tensor("out", [BG, 1184], F32, kind="ExternalOutput")

    # internal DRAM (collective bounce buffers); u_sq ships as fp8 (x8)
    usq_send = nc.dram_tensor("usq_send", [NCORES, BL, RI], F8)
    usq_recv = nc.dram_tensor("usq_recv", [NCORES, BL, RI], F8)
    CCN = BG * CO + 16  # 16656
    ccw_in = nc.dram_tensor("ccw_in", [16], F16)
    ccw_out = nc.dram_tensor("ccw_out", [16], F16, addr_space="Shared")
    cc_in = [nc.dram_tensor(f"cc_in{i}", [CCN], F16) for i in range(3)]
    cc_out = [nc.dram_tensor(f"cc_out{i}", [CCN], F16, addr_space="Shared")
              for i in range(3)]
    GROUPS = [list(range(NCORES))]

    t1, a1, b1, a2, b2, t3, a3, b3, a4, b4 = [float(v) for v in PRIM]
    dt1, da1, db1, da2, db2, dt3, da3, db3, da4, db4 = [float(v) for v in DIGIT]

    with tile.TileContext(nc) as tc:
        const = tc.alloc_tile_pool(name="const", bufs=1)
        z16 = const.tile([1, 16], F16)
        nc.gpsimd.memset(z16[:], 0.0)
        nc.gpsimd.dma_start(_ap(ccw_in[:].tensor, 0, [[16, 1], [1, 16]]),
                            z16[:])
        # warm-up collective: absorbs the first-collective barrier while
        # the conv phase runs
        nc.gpsimd.collective_compute(
            "AllReduce", OP.add, replica_groups=GROUPS,
            ins=[ccw_in[:]], outs=[ccw_out[:]])
        ident = const.tile([128, 128], F32)
        make_identity(nc, ident[:])
        ident16 = const.tile([128, 128], F16)
        nc.vector.tensor_copy(ident16[:], ident[:])
        c1b_sb = const.tile([128, 2], F32)
        nc.gpsimd.dma_start(c1b_sb[:], c1b[:, :])
        c2b_sb = const.tile([128, 2], F32)
        nc.gpsimd.dma_start(c2b_sb[:], c2b[:, :])
        comb_sb = const.tile([128, 128], F16)
        nc.gpsimd.dma_start(comb_sb[:], comb[:, :])
        bmask_sb = const.tile([BG, 1], F32)
        nc.gpsimd.dma_start(bmask_sb[:], bmask[:, :])
        ones8 = const.tile([128, 1], F16)
        nc.gpsimd.memset(ones8[:], 0.125)
        ones104 = const.tile([BG, 1], F32)
        nc.gpsimd.memset(ones104[:], 1.0)
        ones_r104 = const.tile([1, BG], F32)
        nc.gpsimd.memset(ones_r104[:], 1.0)
        ones1_16 = const.tile([1, BG], F16)
        nc.gpsimd.memset(ones1_16[:], 1.0)
        negbig = const.tile([128, 1], F32)
        nc.gpsimd.memset(negbig[:], -1e30)
        # zero the unused tail slots read by the fused collectives
        nc.gpsimd.dma_start(
            _ap(cc_in[0][:].tensor, BG * CO, [[16, 1], [1, 16]]), z16[:])
        for it in (1, 2):
            nc.gpsimd.dma_start(
                _ap(cc_in[it][:].tensor, BG * CO + 10, [[6, 1], [1, 6]]),
                z16[0:1, 0:6])

        persist = tc.alloc_tile_pool(name="persist", bufs=1)
        sq = tc.alloc_tile_pool(name="sq", bufs=1)
        rt = tc.alloc_tile_pool(name="routing", bufs=1)
        W16 = rt.tile([128, 16, CO], F16)
        usq_b = rt.tile([128, RI], F16)  # [b, (r', i)]
        usq_T = rt.tile([128, 16, BG], F16)
        b_rep = rt.tile([128, CO], F32)
        nc.gpsimd.memset(b_rep[:], 0.0)
        vj = rt.tile([BG, CO], F32)  # final v_j lives here after it=2

        # x1 in fp8 (scaled x64): [p, icb, ph, pw, h', w', b]
        x1a = persist.tile([128, 2, 2, 2, 12, 12, BL], F8)
        c2w_sb = persist.tile([128, 2, 81, 256], F8)

        dc = tc.alloc_tile_pool(name="dec", bufs=1)

        # startup DMAs in priority order on the sync queue
        with tc.tile_pool(name="conv1", bufs=1) as c1pool:
            r1 = c1pool.tile([81, BL * 576], F16)
            nc.sync.dma_start(r1[:], r1c[:, :])
            c1w_sb = c1pool.tile([81, 256], F16)
            nc.sync.dma_start(c1w_sb[:], c1w[:, :])
            nc.sync.dma_start(
                c2w_sb[:].rearrange("p a b c -> p (a b c)"), c2w8[:, :])
            nc.sync.dma_start(
                W16[:], _ap(wre[:, :].tensor, 0,
                            [[CO, 128], [128 * CO, 16], [1, CO]]))

            # decoder weights (stream under the conv phase)
            dwsb = {}
            for nm, (kdim, ndim, win_dram) in (
                    ("1", (160, 512, d1)),
                    ("2", (512, 1024, d2)),
                    ("3", (1024, 1024, d3))):
                nkt = (kdim + 127) // 128
                wsb = dc.tile([128, nkt, ndim], F16, tag=f"w{nm}",
                              name=f"w{nm}")
                for kt in range(nkt):
                    ksz = min(128, kdim - kt * 128)
                    nc.sync.dma_start(
                        wsb[:ksz, kt, :],
                        _ap(win_dram[:, :].tensor, kt * 128 * ndim,
                            [[ndim, ksz], [1, ndim]]))
                dwsb[nm] = wsb
            d1b_sb = dc.tile([128, 4], F32)
            nc.sync.dma_start(d1b_sb[:], d1b[:, :])
            d2b_sb = dc.tile([128, 8], F32)
            nc.sync.dma_start(d2b_sb[:], d2b[:, :])
            d3br_sb = dc.tile([1, 1024], F16)
            nc.sync.dma_start(d3br_sb[:], d3br[:, :])

            # ====== conv1: r1c -> x1 fp8 [icb, ph, pw, h', w', b] ======
            with tc.tile_pool(name="c1psum", bufs=2, space="PSUM") as c1ps:
                NTOT = BL * 576  # 7488 per m
                for m in range(2):
                    off = 0
                    while off < NTOT:
                        csz = min(512, NTOT - off)
                        ps = c1ps.tile([128, 512], F32, tag="c1ps")
                        nc.tensor.matmul(ps[:, :csz],
                                         c1w_sb[0:81, m * 128:(m + 1) * 128],
                                         r1[0:81, off:off + csz])
                        xh = x1a[:, m].rearrange(
                            "p a c h w b -> p (a c h w b)")[:, off:off + csz]
                        nc.scalar.activation(xh, ps[:, :csz],
                                             ACT.Relu, bias=c1b_sb[:, m:m + 1])
                        off += csz

        # ============ conv2: fp8 DoubleRow, 4 chunk groups ============
        u_t = [persist.tile([128, BL, 8, 8], F32, tag=f"u_{m}",
                            name=f"u_{m}") for m in range(2)]
        hmax = sq.tile([128, 2, BL], F32)    # [c, m, b]
        hneg = sq.tile([128, 2, BL], F32)
        with tc.tile_pool(name="c2psum", bufs=2, space="PSUM") as c2ps:
            for m in range(2):
                for hc in range(2):
                    ps = c2ps.tile([128, 4, 104], F32, tag="c2ps")
                    for j in range(81):
                        kh, kw = divmod(j, 9)
                        ph, h0 = kh & 1, kh >> 1
                        pw, w0 = kw & 1, kw >> 1
                        rhs = _ap(x1a[:].tensor,
                                  x1a[:].offset + ph * 3744 + pw * 1872
                                  + (hc * 4 + h0) * 156 + w0 * 13,
                                  [list(x1a[:].ap[0]), [7488, 2],
                                   [156, 4], [1, 104]])
                        nc.tensor.matmul(ps[:], c2w_sb[:, :, j,
                                                       m * 128:(m + 1) * 128],
                                         rhs, start=(j == 0), stop=(j == 80),
                                         perf_mode=DR)
                    # readout: psum [p, oh(4), w(8), b(13)] -> u_t [p,b,oh,w]
                    pst = ps[:]
                    src = _ap(pst.tensor, pst.offset,
                              [list(pst.ap[0]), [1, BL], [104, 4], [13, 8]])
                    ut = u_t[m][:]
                    dst = _ap(ut.tensor, ut.offset + hc * 4 * 8,
                              [list(ut.ap[0]), [64, BL], [8, 4], [1, 8]])
                    nc.scalar.activation(dst, src, ACT.Identity,
                                         bias=c2b_sb[:, m:m + 1],
                                         scale=1.0 / (XS * WS))
                    # incremental squash maxima (hidden under next chunk)
                    xs = u_t[m][:, :, hc * 4:hc * 4 + 4, 0]  # [128, b, 4h]
                    red = sq.tile([128, BL], F32, tag="red")
                    nc.vector.tensor_reduce(red[:], xs, AX.X, OP.max)
                    if hc == 0:
                        nc.vector.tensor_copy(hmax[:, m, :], red[:])
                    else:
                        nc.vector.tensor_tensor(hmax[:, m, :], hmax[:, m, :],
                                                red[:], OP.max)
                    msk = sq.tile([128, BL, 4], I32, tag="msk")
                    nc.vector.tensor_single_scalar(msk[:], xs, 0.0, OP.is_lt)
                    xn = sq.tile([128, BL, 4], F32, tag="xn")
                    nc.vector.tensor_copy(
                        xn[:], negbig[:, 0:1].to_broadcast((128, BL, 4)))
                    nc.vector.copy_predicated(xn[:], msk[:], xs)
                    nc.vector.tensor_reduce(red[:], xn[:], AX.X, OP.max)
                    if hc == 0:
                        nc.vector.tensor_copy(hneg[:, m, :], red[:])
                    else:
                        nc.vector.tensor_tensor(hneg[:, m, :], hneg[:, m, :],
                                                red[:], OP.max)

        # ======== primary squash (value-threshold form) + u_sq ========
        # cross-partition max, replicated to all partitions
        redM = sq.tile([128, 2 * BL], F32)
        redN = sq.tile([128, 2 * BL], F32)
        hmax2 = hmax[:].rearrange("p m b -> p (m b)")
        hneg2 = hneg[:].rearrange("p m b -> p (m b)")
        nc.gpsimd.partition_all_reduce(redM[:], hmax2, channels=128,
                                       reduce_op=bass_isa.ReduceOp.max)
        nc.gpsimd.partition_all_reduce(redN[:], hneg2, channels=128,
                                       reduce_op=bass_isa.ReduceOp.max)
        Mb = sq.tile([128, BL], F32)
        Nb = sq.tile([128, BL], F32)
        nc.vector.tensor_tensor(Mb[:], redM[:, 0:BL],
                                redM[:, BL:2 * BL], OP.max)
        nc.vector.tensor_tensor(Nb[:], redN[:, 0:BL],
                                redN[:, BL:2 * BL], OP.max)

        usq = [persist.tile([128, BL, 8, 8], F8, tag=f"usq_{m}",
                            name=f"usq_{m}") for m in range(2)]
        for m in range(2):
            xs = u_t[m][:, :, :, 0]          # [128, b, h]
            y = sq.tile([128, BL, 8], F32, tag="y")
            aff = sq.tile([128, BL, 8], F32, tag="aff")
            mk = sq.tile([128, BL, 8], I32, tag="mk")
            mk2 = sq.tile([128, BL, 8], I32, tag="mk2")
            nc.vector.tensor_copy(y[:], xs)
            # x < mneg -> a2*x+b2
            nc.vector.tensor_tensor(
                mk[:], xs, Nb[:, :, None].to_broadcast((128, BL, 8)),
                OP.is_lt)
            nc.vector.tensor_scalar(aff[:], xs, a2, b2, OP.mult, OP.add)
            nc.vector.copy_predicated(y[:], mk[:], aff[:])
            # (x >= 0) & (x < M) -> a3*x+b3
            nc.vector.tensor_single_scalar(mk[:], xs, 0.0, OP.is_ge)
            nc.vector.tensor_tensor(
                mk2[:], xs, Mb[:, :, None].to_broadcast((128, BL, 8)),
                OP.is_lt)
            nc.vector.tensor_tensor(mk[:], mk[:], mk2[:], OP.mult)
            nc.vector.tensor_scalar(aff[:], xs, a3, b3, OP.mult, OP.add)
            nc.vector.copy_predicated(y[:], mk[:], aff[:])
            # u_sq_fp8 = 8 * y * u  (x8 for fp8 range; undone on receive)
            nc.vector.tensor_scalar(y[:], y[:], 8.0, None, OP.mult)
            nc.vector.tensor_tensor(
                usq[m][:], u_t[m][:],
                y[:, :, :, None].to_broadcast((128, BL, 8, 8)), OP.mult)

        # scatter to send buffer [dest][b][(ch'=m*16+c)*64 + oh*8 + i]
        engs = [nc.sync, nc.scalar, nc.gpsimd]
        for m in range(2):
            for d in range(NCORES):
                dst = _ap(usq_send[:].tensor, d * (BL * RI) + m * 1024,
                          [[64, 16], [RI, BL], [1, 64]])
                engs[(m * NCORES + d) % 3].dma_start(
                    dst, usq[m][d * 16:(d + 1) * 16, :, :, :])

        # ============ AllToAll: u_sq -> route-sharded, full batch ========
        nc.gpsimd.collective_compute(
            "AllToAll", OP.bypass, replica_groups=GROUPS,
            ins=[usq_send[:]], outs=[usq_recv[:]])

        # ============ routing ============
        usq8_b = rt.tile([128, RI], F8)
        nc.sync.dma_start(
            usq8_b[0:BG, :], _ap(usq_recv[:].tensor, 0, [[RI, BG], [1, RI]]))
        with tc.tile_pool(name="tps", bufs=2, space="PSUM") as tps:
            # p-state warmers: keep the PE clocked up through the AllToAll
            psW = tps.tile([128, BG], F32, tag="psW")
            for dk in range(40):
                nc.tensor.matmul(psW[:], ident16[:, 0:128],
                                 ident16[:, 0:BG], start=True, stop=True)
            # un-scale the fp8 payload back to fp16 u_sq
            nc.vector.tensor_scalar(usq_b[0:BG, :], usq8_b[0:BG, :],
                                    0.125, None, OP.mult)
            for t in range(16):
                pt = tps.tile([128, BG], F16, tag="pt")
                nc.tensor.transpose(pt[:], usq_b[0:BG, 128 * t:128 * (t + 1)],
                                    ident16[0:BG, 0:BG])
                nc.vector.tensor_copy(usq_T[:, t, :], pt[:])

        with tc.tile_pool(name="rloop", bufs=3) as rl, \
             tc.tile_pool(name="rpsS", bufs=1, space="PSUM") as rpsS, \
             tc.tile_pool(name="rpsT", bufs=1, space="PSUM") as rpsT, \
             tc.tile_pool(name="rps1", bufs=1, space="PSUM") as rps1:
            for it in range(3):
                if it == 0:
                    # b_ij = 0: c = W, E_c = 2048 exactly
                    mc = W16
                else:
                    cexp = rl.tile([128, CO], F16, tag="cexp")
                    nc.scalar.activation(cexp[:], b_rep[:], ACT.Exp)
                    mc = rl.tile([128, 16, CO], F16, tag="mc")
                    cexp_b = _ap(cexp[:].tensor, cexp[:].offset,
                                 [list(cexp[:].ap[0]), [10, 16], [1, 10],
                                  [0, 16]])
                    nc.vector.tensor_tensor(
                        mc[:].rearrange("p t (c o) -> p t c o", c=10),
                        W16[:].rearrange("p t (c o) -> p t c o", c=10),
                        cexp_b, OP.mult)
                    # E_c partial
                    psE = rps1.tile([1, CO], F32, tag="psE")
                    nc.tensor.matmul(psE[:], ones8[:], cexp[:])
                    E10 = rl.tile([1, 10], F16, tag="E10")
                    psE_v = _ap(psE[:].tensor, psE[:].offset,
                                [list(psE[:].ap[0]), [1, 10], [10, 16]])
                    with nc.allow_low_precision(
                            reason="E sums ~2048, fp16 ok (validated)"):
                        nc.vector.tensor_reduce(E10[:], psE_v, AX.X, OP.add)
                # s_tilde
                psS = rpsS.tile([BG, CO], F32, tag="psS")
                for t in range(16):
                    nc.tensor.matmul(psS[:], usq_T[:, t, :], mc[:, t, :],
                                     start=(t == 0), stop=(t == 15))
                s_sb = rl.tile([BG, CO], F16, tag="s_sb")
                nc.vector.tensor_copy(s_sb[:], psS[:])
                nc.sync.dma_start(
                    _ap(cc_in[it][:].tensor, 0, [[CO, BG], [1, CO]]), s_sb[:])
                if it > 0:
                    nc.sync.dma_start(
                        _ap(cc_in[it][:].tensor, BG * CO, [[1, 1], [1, 10]]),
                        E10[:])
                # p-state warmers: keep the PE clocked up through the
                # collective so post-AR matmuls run at full rate
                for dk in range(36):
                    nc.tensor.matmul(psS[:, 0:BG], usq_T[:, 0, :],
                                     usq_T[:, 1, :], start=True, stop=True)
                nc.gpsimd.collective_compute(
                    "AllReduce", OP.add, replica_groups=GROUPS,
                    ins=[cc_in[it][:]], outs=[cc_out[it][:]])
                s_full = rl.tile([BG, CO], F16, tag="s_full")
                nc.sync.dma_start(
                    s_full[:],
                    _ap(cc_out[it][:].tensor, 0, [[CO, BG], [1, CO]]))
                sj = rl.tile([BG, CO], F32, tag="sj")
                if it == 0:
                    nc.vector.tensor_scalar(sj[:], s_full[:], 1.0 / 2048.0,
                                            None, OP.mult)
                else:
                    sf32 = rl.tile([BG, CO], F32, tag="sf32")
                    nc.vector.tensor_copy(sf32[:], s_full[:])
                    E10r = rl.tile([1, 10], F16, tag="E10r")
                    nc.sync.dma_start(
                        E10r[:],
                        _ap(cc_out[it][:].tensor, BG * CO, [[1, 1], [1, 10]]))
                    E32 = rl.tile([1, 10], F32, tag="E32")
                    nc.vector.tensor_copy(E32[:], E10r[:])
                    rE = rl.tile([1, 10], F32, tag="rE")
                    nc.vector.reciprocal(rE[:], E32[:])
                    psBE = rps1.tile([BG, CO], F32, tag="psBE")
                    rE_b = _ap(rE[:].tensor, rE[:].offset,
                               [list(rE[:].ap[0]), [1, 10], [0, 16]])
                    nc.tensor.matmul(psBE[:], ones_r104[:], rE_b)
                    nc.vector.tensor_tensor(sj[:], sf32[:], psBE[:],
                                            OP.mult)

                # ---- digit squash (exact rank arithmetic) ----
                x10 = _ap(sj[:].tensor, sj[:].offset,
                          [list(sj[:].ap[0]), [16, 10]])
                cmp = rl.tile([BG, 10, 10], F32, tag="cmp")
                x_j = _ap(sj[:].tensor, sj[:].offset,
                          [list(sj[:].ap[0]), [16, 10], [0, 10]])
                x_k = _ap(sj[:].tensor, sj[:].offset,
                          [list(sj[:].ap[0]), [0, 10], [16, 10]])
                nc.vector.tensor_tensor(cmp[:], x_j, x_k, OP.is_gt)
                r10 = rl.tile([BG, 10], F32, tag="r10")
                nc.vector.tensor_reduce(r10[:], cmp[:], AX.X, OP.add)
                y = rl.tile([BG, 10], F32, tag="y")
                tmp = rl.tile([BG, 10], F32, tag="tmp")
                aff = rl.tile([BG, 10], F32, tag="aff")
                mkA = rl.tile([BG, 10], I32, tag="mkA")
                mkB = rl.tile([BG, 10], I32, tag="mkB")
                cnt = rl.tile([BG, 4], F32, tag="cnt")  # i1, i2, i3 columns
                # i1
                nc.vector.tensor_single_scalar(tmp[:], x10, dt1, OP.is_lt)
                nc.vector.tensor_reduce(cnt[:, 0:1], tmp[:], AX.X, OP.add)
                # stage 1: r < i1 - 1
                nc.vector.tensor_copy(y[:], x10)
                nc.vector.tensor_scalar(tmp[:], cnt[:, 0:1].to_broadcast(
                    (BG, 10)), 1.0, None, OP.subtract)
                nc.vector.tensor_tensor(mkA[:], r10[:], tmp[:], OP.is_lt)
                nc.vector.tensor_scalar(aff[:], x10, da1, db1, OP.mult, OP.add)
                nc.vector.copy_predicated(y[:], mkA[:], aff[:])
                # i2 on modified y
                nc.vector.tensor_single_scalar(tmp[:], y[:], 0.0, OP.is_lt)
                nc.vector.tensor_reduce(cnt[:, 1:2], tmp[:], AX.X, OP.add)
                # stage 2: (r >= i1) & (r < i2 - 1)
                nc.vector.tensor_tensor(
                    mkA[:], r10[:], cnt[:, 0:1].to_broadcast((BG, 10)),
                    OP.is_ge)
                nc.vector.tensor_scalar(tmp[:], cnt[:, 1:2].to_broadcast(
                    (BG, 10)), 1.0, None, OP.subtract)
                nc.vector.tensor_tensor(mkB[:], r10[:], tmp[:], OP.is_lt)
                nc.vector.tensor_tensor(mkA[:], mkA[:], mkB[:], OP.mult)
                nc.vector.tensor_scalar(aff[:], y[:], da2, db2, OP.mult, OP.add)
                nc.vector.copy_predicated(y[:], mkA[:], aff[:])
                # i3 on modified y
                nc.vector.tensor_single_scalar(tmp[:], y[:], dt3, OP.is_lt)
                nc.vector.tensor_reduce(cnt[:, 2:3], tmp[:], AX.X, OP.add)
                # stage 3: (r >= i2) & (r < i3 - 1)
                nc.vector.tensor_tensor(
                    mkA[:], r10[:], cnt[:, 1:2].to_broadcast((BG, 10)),
                    OP.is_ge)
                nc.vector.tensor_scalar(tmp[:], cnt[:, 2:3].to_broadcast(
                    (BG, 10)), 1.0, None, OP.subtract)
                nc.vector.tensor_tensor(mkB[:], r10[:], tmp[:], OP.is_lt)
                nc.vector.tensor_tensor(mkA[:], mkA[:], mkB[:], OP.mult)
                nc.vector.tensor_scalar(aff[:], y[:], da3, db3, OP.mult, OP.add)
                nc.vector.copy_predicated(y[:], mkA[:], aff[:])
                # stage 4: (r >= i3) & (r < 9)
                nc.vector.tensor_tensor(
                    mkA[:], r10[:], cnt[:, 2:3].to_broadcast((BG, 10)),
                    OP.is_ge)
                nc.vector.tensor_single_scalar(mkB[:], r10[:], 9.0, OP.is_lt)
                nc.vector.tensor_tensor(mkA[:], mkA[:], mkB[:], OP.mult)
                nc.vector.tensor_scalar(aff[:], y[:], da4, db4, OP.mult, OP.add)
                nc.vector.copy_predicated(y[:], mkA[:], aff[:])
                # v_j = f * s_mod (s_mod[:, :, 0] = f)
                if it == 2:
                    vdst = vj
                else:
                    vdst = rl.tile([BG, CO], F32, tag="vtmp", name="vtmp")
                nc.vector.tensor_copy(vdst[:], sj[:])
                vdst0 = _ap(vdst[:].tensor, vdst[:].offset,
                            [list(vdst[:].ap[0]), [16, 10]])
                nc.vector.tensor_copy(vdst0, y[:])
                f_b = _ap(y[:].tensor, y[:].offset,
                          [list(y[:].ap[0]), [1, 10], [0, 16]])
                nc.vector.tensor_tensor(
                    vdst[:].rearrange("b (c o) -> b c o", c=10),
                    vdst[:].rearrange("b (c o) -> b c o", c=10), f_b, OP.mult)

                if it < 2:
                    v16 = rl.tile([BG, CO], F16, tag="v16", name="v16")
                    nc.vector.tensor_copy(v16[:], vdst[:])
                    qall = rl.tile([128, 16, 10], F16, tag="qall")
                    for half in range(2):
                        psT = rpsT.tile([128, 8, 256], F32, tag="psT")
                        for j in range(8):
                            t = half * 8 + j
                            nc.tensor.matmul(
                                psT[:, j, 0:CO],
                                usq_b[0:BG, 128 * t:128 * (t + 1)], v16[:])
                        T16 = rl.tile([128, 8, CO], F16, tag="T16")
                        nc.vector.tensor_copy(T16[:], psT[:, :, 0:CO])
                        prod = rl.tile([128, 8, CO], F16, tag="prod")
                        nc.vector.tensor_tensor(
                            prod[:], W16[:, 8 * half:8 * (half + 1), :],
                            T16[:], OP.mult)
                        with nc.allow_low_precision(
                                reason="16-term o-sum feeding small logits"):
                            nc.vector.tensor_reduce(
                                qall[:, 8 * half:8 * (half + 1), :],
                                prod[:].rearrange("p j (c o) -> p j c o",
                                                  c=10),
                                AX.X, OP.add)
                    psA = rpsS.tile([128, CO], F32, tag="psA")
                    nc.tensor.matmul(psA[:], comb_sb[:],
                                     qall[:].rearrange("p t c -> p (t c)"))
                    nc.vector.tensor_tensor(b_rep[:], b_rep[:], psA[:], OP.add)

        # ============ decoder (all 104 rows, identical on every core) ====
        with tc.tile_pool(name="dps", bufs=2, space="PSUM") as dps:
            sqv = dc.tile([BG, CO], F32)
            nc.scalar.activation(sqv[:], vj[:], ACT.Square)
            csum = dc.tile([BG, 10], F32)
            sq_v = _ap(sqv[:].tensor, sqv[:].offset,
                       [list(sqv[:].ap[0]), [16, 10], [1, 16]])
            nc.vector.tensor_reduce(csum[:], sq_v, AX.X, OP.add)
            classes = dc.tile([BG, 10], F32)
            nc.scalar.activation(classes[:], csum[:], ACT.Sqrt)
            expcl = dc.tile([BG, 10], F32)
            nc.scalar.activation(expcl[:], classes[:], ACT.Exp)
            nc.vector.tensor_scalar_mul(expcl[:], expcl[:], bmask_sb[:, 0:1])
            psD = dps.tile([10, 1], F32, tag="dsmall")
            nc.tensor.matmul(psD[:], expcl[:], ones104[:])
            dsb = dc.tile([10, 1], F32)
            nc.vector.tensor_copy(dsb[:], psD[:])
            psDT = dps.tile([1, 10], F32, tag="dsmall")
            nc.tensor.transpose(psDT[:], dsb[:], ident[0:10, 0:10])
            dT = dc.tile([1, 10], F32)
            nc.vector.tensor_copy(dT[:], psDT[:])
            rD = dc.tile([1, 10], F32)
            nc.vector.reciprocal(rD[:], dT[:])
            psBD = dps.tile([BG, 10], F32, tag="dsmall")
            rD_b = _ap(rD[:].tensor, rD[:].offset,
                       [list(rD[:].ap[0]), [1, 10]])
            nc.tensor.matmul(psBD[:], ones_r104[:], rD_b)
            p = dc.tile([BG, 10], F32)
            nc.vector.tensor_tensor(p[:], expcl[:], psBD[:], OP.mult)
            pm = dc.tile([BG, 1], F32)
            nc.vector.tensor_reduce(pm[:], p[:], AX.X, OP.max)
            mask = dc.tile([BG, 10], F32)
            nc.vector.tensor_tensor(mask[:], p[:],
                                    pm[:].to_broadcast((BG, 10)), OP.is_ge)
            tm = dc.tile([BG, CO], F16)
            mask_b = _ap(mask[:].tensor, mask[:].offset,
                         [list(mask[:].ap[0]), [1, 10], [0, 16]])
            nc.vector.tensor_tensor(
                tm[:].rearrange("b (c o) -> b c o", c=10),
                vj[:].rearrange("b (c o) -> b c o", c=10), mask_b, OP.mult)
            nc.sync.dma_start(out[:, 0:160], vj[:])

            # tT [160, 104] via PE transposes
            tT = dc.tile([128, 2, BG], F16)
            for kt in range(2):
                ksz = 128 if kt == 0 else 32
                pst = dps.tile([128, BG], F16, tag="dpst")
                nc.tensor.transpose(pst[:ksz, :],
                                    tm[:, kt * 128:kt * 128 + ksz],
                                    ident16[0:BG, 0:BG])
                nc.vector.tensor_copy(tT[:ksz, kt, :], pst[:ksz, :])

            # L1/L2 weight-stationary -> transposed activations
            h1T = dc.tile([128, 4, BG], F16)
            for mt in range(4):
                psH = dps.tile([128, BG], F32, tag="dpsH")
                for kt in range(2):
                    ksz = 128 if kt == 0 else 32
                    nc.tensor.matmul(
                        psH[:], dwsb["1"][:ksz, kt, mt * 128:(mt + 1) * 128],
                        tT[:ksz, kt, :], start=(kt == 0), stop=(kt == 1))
                nc.scalar.activation(h1T[:, mt, :], psH[:], ACT.Relu,
                                     bias=d1b_sb[:, mt:mt + 1])
            h2T = dc.tile([128, 8, BG], F16)
            for mt in range(8):
                psH = dps.tile([128, BG], F32, tag="dpsH")
                for kt in range(4):
                    nc.tensor.matmul(
                        psH[:], dwsb["2"][:, kt, mt * 128:(mt + 1) * 128],
                        h1T[:, kt, :], start=(kt == 0), stop=(kt == 3))
                nc.scalar.activation(h2T[:, mt, :], psH[:], ACT.Relu,
                                     bias=d2b_sb[:, mt:mt + 1])
            # L3 moving-form: r3 [104, 1024] batch-major, bias via K=1 row
            r3 = dc.tile([BG, 1024], F32)
            for half in range(2):
                psR = dps.tile([BG, 512], F32, tag="dpsR")
                for kt in range(8):
                    nc.tensor.matmul(
                        psR[:], h2T[:, kt, :],
                        dwsb["3"][:, kt, half * 512:(half + 1) * 512],
                        start=(kt == 0), stop=False)
                nc.tensor.matmul(
                    psR[:], ones1_16[:],
                    d3br_sb[0:1, half * 512:(half + 1) * 512],
                    start=False, stop=True)
                nc.scalar.activation(r3[:, half * 512:(half + 1) * 512],
                                     psR[:], ACT.Sigmoid)
            nc.sync.dma_start(out[:, 160:1184], r3[:])

        dc.release()
        rt.release()
        sq.release()
        persist.release()
        const.release()

    nc.compile()
    return nc


_PROGRAM = None


def _get_program():
    global _PROGRAM
    if _PROGRAM is None:
        _PROGRAM = build_program()
    return _PROGRAM


def _prepare_in_maps(inputs):
    data = np.asarray(inputs["data"], dtype=np.float32)      # (100,1,32,32)
    conv1_w = np.asarray(inputs["conv1_w"], dtype=np.float32)
    conv1_b = np.asarray(inputs["conv1_b"], dtype=np.float32)
    prim_w = np.asarray(inputs["prim_w"], dtype=np.float32)
    prim_b = np.asarray(inputs["prim_b"], dtype=np.float32)
    W_dc = np.asarray(inputs["W_dc"], dtype=np.float32)
    dec_w1 = np.asarray(inputs["dec_w1"], dtype=np.float32)
    dec_b1 = np.asarray(inputs["dec_b1"], dtype=np.float32)
    dec_w2 = np.asarray(inputs["dec_w2"], dtype=np.float32)
    dec_b2 = np.asarray(inputs["dec_b2"], dtype=np.float32)
    dec_w3 = np.asarray(inputs["dec_w3"], dtype=np.float32)
    dec_b3 = np.asarray(inputs["dec_b3"], dtype=np.float32)

    B = data.shape[0]
    data_pad = np.zeros((BG, 32, 32), np.float32)
    data_pad[:B] = data[:, 0]
    swv = np.lib.stride_tricks.sliding_window_view(data_pad, (24, 24),
                                                   axis=(1, 2))
    # swv[b, kh, kw, oh, ow] = data[b, oh+kh, ow+kw]
    # columns (ph=oh&1, pw=ow&1, h'=oh>>1, w'=ow>>1, b)
    t5 = swv.transpose(1, 2, 0, 3, 4).reshape(81, BG, 12, 2, 12, 2)
    r1c_all = np.ascontiguousarray(
        t5.transpose(0, 3, 5, 2, 4, 1)).astype(np.float16)  # [81,ph,pw,h,w,b]

    c1w = np.ascontiguousarray(
        conv1_w.transpose(2, 3, 1, 0).reshape(81, 256)).astype(np.float16)
    c1w = (c1w.astype(np.float32) * XS).astype(np.float16)
    c1b = np.zeros((128, 2), np.float32)
    c1b[:, 0] = conv1_b[:128] * XS
    c1b[:, 1] = conv1_b[128:] * XS
    # conv2 weights: [p, icb, tap, oc] * WS -> fp8
    c2w8 = np.ascontiguousarray(
        prim_w.transpose(1, 2, 3, 0).reshape(2, 128, 81, 256)
        .transpose(1, 0, 2, 3)).reshape(128, 2 * 81 * 256)
    c2w8 = (c2w8 * WS).astype(ml_dtypes.float8_e4m3fn)
    c2b = np.zeros((128, 2), np.float32)
    c2b[:, 0] = prim_b[:128]
    c2b[:, 1] = prim_b[128:]
    comb = np.zeros((128, 128), np.float16)
    for blk in range(16):
        comb[blk * 8:(blk + 1) * 8, blk * 8:(blk + 1) * 8] = 0.01
    d1 = np.ascontiguousarray(dec_w1.T).astype(np.float16)
    d1b = np.ascontiguousarray(dec_b1.reshape(4, 128).T)
    d2 = np.ascontiguousarray(dec_w2.T).astype(np.float16)
    d2b = np.ascontiguousarray(dec_b2.reshape(8, 128).T)
    d3 = np.ascontiguousarray(dec_w3.T).astype(np.float16)
    d3br = np.ascontiguousarray(dec_b3.reshape(1, 1024)).astype(np.float16)
    bm = np.zeros((BG, 1), np.float32)
    bm[:B] = 1.0

    # route shard: core k, r' = (ch', oh), ch' = m*16 + c_loc,
    # global ch = m*128 + k*16 + c_loc
    rp = np.arange(256)
    chp = rp >> 3
    oh = rp & 7
    m_ = chp >> 4
    c_loc = chp & 15

    in_maps = []
    for k in range(NCORES):
        gch = m_ * 128 + k * 16 + c_loc
        gr = gch * 8 + oh                       # global route index
        # wre rows (r', i): [256, 8, 160] from W_dc[gr] [10, 16, 8]
        wk = W_dc[gr]                           # [256, 10, 16, 8]
        wre = np.ascontiguousarray(
            wk.transpose(0, 3, 1, 2).reshape(RI, CO)).astype(np.float16)
        in_maps.append({
            "r1c": np.ascontiguousarray(
                r1c_all[:, :, :, :, :, k * BL:(k + 1) * BL]
                .reshape(81, BL * 576)),
            "c1w": c1w, "c1b": c1b, "c2w8": c2w8, "c2b": c2b,
            "wre": wre, "comb": comb, "bmask": bm,
            "d1": d1, "d1b": d1b, "d2": d2, "d2b": d2b,
            "d3": d3, "d3br": d3br,
        })

    return in_maps, B


def kernel(**inputs):
    in_maps, B = _prepare_in_maps(inputs)
    nc = _get_program()
    res = run_bass_kernel_spmd(nc, in_maps, list(range(NCORES)))
    return res.results[0]["out"][:B]


def timed_run(inputs):
    in_maps, _ = _prepare_in_maps(inputs)
    nc = _get_program()
    res = run_bass_kernel_spmd(nc, in_maps, list(range(NCORES)), trace=True)
    if res.exec_time_ns is None:
        raise RuntimeError("exec_time_ns unavailable")
    return res.exec_time_ns


# revision 41
# speedup vs baseline: 1.1180x; 1.0429x over previous
"""CapsNet forward on 8 Trainium2 NeuronCores (Bass/Tile).

Strategy (v2):
  - conv1 (9x9 s1 + relu) as im2col matmul in fp16, writing x1 in fp8
    (x64 scale) with layout [p, icb, ph, pw, h', w', b].
  - conv2 (9x9 s2) in fp8e4m3 with DoubleRow perf mode: K-pairs over the
    two input-channel blocks; moving operand merges (w-window x batch)
    into one contiguous dim; 4 chunk-major accumulation groups (m, oh-half).
  - primary squash (value-threshold form) + u_sq = mag * u in fp16.
  - single AllToAll (fp16) to route-parallel: dest d owns channels
    {m*128 + d*16 + c : c<16, m in 0,1}; payload [dest][b][r', i] so the
    receive side is one uniform-stride DMA.
  - routing (3 iters): s_j via [(r,i) x b]^T @ (exp(b_ij) . W) fp16
    matmuls, fused AllReduce carrying [s_tilde | sum_exp]; agreement via
    T = u_sq^T v + comb-matmul; digit squash exact rank arithmetic fp32.
  - decoder computed for all 104 rows on every core (identical results);
    L1/L2 weight-stationary (bias fused per-partition), L3 moving-form
    with bias as a K=1 matmul row; core 0's output is used by the host.
"""

import numpy as np
import ml_dtypes

import concourse.bass as bass
import concourse.mybir as mybir
import concourse.tile as tile
from concourse import bacc
from concourse.bass_utils import run_bass_kernel_spmd
from concourse.masks import make_identity
from concourse import bass_isa

F32 = mybir.dt.float32
I32 = mybir.dt.int32
F16 = mybir.dt.float16
F8 = mybir.dt.float8e4
AX = mybir.AxisListType
OP = mybir.AluOpType
ACT = mybir.ActivationFunctionType
DR = mybir.MatmulPerfMode.DoubleRow

NCORES = 8
BL = 13            # batch rows per core
BG = NCORES * BL   # 104 (padded batch)
NR, NC_, DI, DO = 2048, 10, 8, 16
RSH = NR // NCORES  # 256 routes per core
CO = NC_ * DO       # 160
RI = RSH * DI       # 2048 = (r', i) per core
XS = 32.0           # x1 fp8 scale (TRN2 fp8e4 saturates at 240)
WS = 4096.0         # conv2 weight fp8 scale
US = 8.0            # u_sq fp8 transport scale
WS2 = 512.0         # W_dc fp8 scale (iter-0 local s_j)

PRIM = (-13.46416092, 0.000242759, 0.024488359, 0.002769205, 0.06089699,
        13.23405266, -0.002828244, 0.061313814, -0.000219038, 0.023874787)
DIGIT = (-0.075410217, -0.074520095, 0.349297946, -0.534473989, 0.27196494,
         0.062207676, 0.637642944, 0.295330779, 0.169344703, 0.353784456)


def _ap(t, offset, dims):
    return bass.AP(tensor=t, offset=offset, ap=[list(d) for d in dims])


def build_program():
    nc = bacc.Bacc("TRN2", target_bir_lowering=False, debug=False,
                   num_devices=NCORES)

    # ---------------- I/O ----------------
    r1c = nc.dram_tensor("r1c", [81, BL * 576], F16, kind="ExternalInput")
    c1w = nc.dram_tensor("c1w", [81, 256], F16, kind="ExternalInput")
    c1b = nc.dram_tensor("c1b", [128, 2], F32, kind="ExternalInput")
    c2w8 = nc.dram_tensor("c2w8", [128, 2 * 81 * 256], F8,
                          kind="ExternalInput")
    c2b = nc.dram_tensor("c2b", [128, 2], F32, kind="ExternalInput")
    wre = nc.dram_tensor("wre", [RI, CO], F16, kind="ExternalInput")
    wf3 = nc.dram_tensor("wf3", [128, 2 * 64 * CO], F8, kind="ExternalInput")
    comb = nc.dram_tensor("comb", [128, 128], F16, kind="ExternalInput")
    bmask = nc.dram_tensor("bmask", [BG, 1], F32, kind="ExternalInput")
    d1 = nc.dram_tensor("d1", [160, 512], F16, kind="ExternalInput")
    d1b = nc.dram_tensor("d1b", [128, 4], F32, kind="ExternalInput")
    d2 = nc.dram_tensor("d2", [512, 1024], F16, kind="ExternalInput")
    d2b = nc.dram_tensor("d2b", [128, 8], F32, kind="ExternalInput")
    d3 = nc.dram_tensor("d3", [1024, 1024], F16, kind="ExternalInput")
    d3br = nc.dram_tensor("d3br", [1, 1024], F16, kind="ExternalInput")
    out = nc.dram_tensor("out", [BG, 1184], F32, kind="ExternalOutput")

    # internal DRAM (collective bounce buffers); u_sq ships as fp8 (x8)
    usq_send = nc.dram_tensor("usq_send", [NCORES, BL, RI], F8)
    usq_recv = nc.dram_tensor("usq_recv", [NCORES, BL, RI], F8)
    CCN = BG * CO + 16  # 16656
    ccw_in = nc.dram_tensor("ccw_in", [16], F16)
    ccw_out = nc.dram_tensor("ccw_out", [16], F16, addr_space="Shared")
    cc_in = [nc.dram_tensor(f"cc_in{i}", [CCN], F16) for i in (1, 2)]
    cc_in = {1: cc_in[0], 2: cc_in[1]}
    cc_out = {i: nc.dram_tensor(f"cc_out{i}", [CCN], F16,
                                addr_space="Shared") for i in (1, 2)}
    ag_in = nc.dram_tensor("ag_in", [BL * CO], F16)
    ag_out = nc.dram_tensor("ag_out", [BG * CO], F16, addr_space="Shared")
    GROUPS = [list(range(NCORES))]

    t1, a1, b1, a2, b2, t3, a3, b3, a4, b4 = [float(v) for v in PRIM]
    dt1, da1, db1, da2, db2, dt3, da3, db3, da4, db4 = [float(v) for v in DIGIT]

    with tile.TileContext(nc) as tc:
        const = tc.alloc_tile_pool(name="const", bufs=1)
        z16 = const.tile([1, 16], F16)
        nc.gpsimd.memset(z16[:], 0.0)
        nc.gpsimd.dma_start(_ap(ccw_in[:].tensor, 0, [[16, 1], [1, 16]]),
                            z16[:])
        # warm-up collective: absorbs the first-collective barrier while
        # the conv phase runs
        nc.gpsimd.collective_compute(
            "AllReduce", OP.add, replica_groups=GROUPS,
            ins=[ccw_in[:]], outs=[ccw_out[:]])
        ident = const.tile([128, 128], F32)
        make_identity(nc, ident[:])
        ident16 = const.tile([128, 128], F16)
        nc.vector.tensor_copy(ident16[:], ident[:])
        c1b_sb = const.tile([128, 2], F32)
        nc.gpsimd.dma_start(c1b_sb[:], c1b[:, :])
        c2b_sb = const.tile([128, 2], F32)
        nc.gpsimd.dma_start(c2b_sb[:], c2b[:, :])
        comb_sb = const.tile([128, 128], F16)
        nc.gpsimd.dma_start(comb_sb[:], comb[:, :])
        bmask_sb = const.tile([BG, 1], F32)
        nc.gpsimd.dma_start(bmask_sb[:], bmask[:, :])
        ones8 = const.tile([128, 1], F16)
        nc.gpsimd.memset(ones8[:], 0.125)
        ones104 = const.tile([BG, 1], F32)
        nc.gpsimd.memset(ones104[:], 1.0)
        ones_r104 = const.tile([1, BG], F32)
        nc.gpsimd.memset(ones_r104[:], 1.0)
        ones1_16 = const.tile([1, BG], F16)
        nc.gpsimd.memset(ones1_16[:], 1.0)
        negbig = const.tile([128, 1], F32)
        nc.gpsimd.memset(negbig[:], -1e30)
        # zero the unused tail slots read by the fused collectives
        for it in (1, 2):
            nc.gpsimd.dma_start(
                _ap(cc_in[it][:].tensor, BG * CO + 10, [[6, 1], [1, 6]]),
                z16[0:1, 0:6])

        persist = tc.alloc_tile_pool(name="persist", bufs=1)
        sq = tc.alloc_tile_pool(name="sq", bufs=1)
        rt = tc.alloc_tile_pool(name="routing", bufs=1)
        W16 = rt.tile([128, 16, CO], F16)
        usq_b = rt.tile([128, RI], F16)  # [b, (r', i)]
        usq_T = rt.tile([128, 16, BG], F16)
        b_rep = rt.tile([128, CO], F32)
        nc.gpsimd.memset(b_rep[:], 0.0)
        vj = rt.tile([BG, CO], F32)  # final v_j lives here after it=2

        # x1 in fp8 (scaled x64): [p, icb, ph, pw, h', w', b]
        x1a = persist.tile([128, 2, 2, 2, 12, 12, BL], F8)
        c2w_sb = persist.tile([128, 2, 81, 256], F8)

        dc = tc.alloc_tile_pool(name="dec", bufs=1)

        # startup DMAs in priority order on the sync queue
        with tc.tile_pool(name="conv1", bufs=1) as c1pool:
            r1 = c1pool.tile([81, BL * 576], F16)
            nc.sync.dma_start(r1[:], r1c[:, :])
            c1w_sb = c1pool.tile([81, 256], F16)
            nc.sync.dma_start(c1w_sb[:], c1w[:, :])
            nc.sync.dma_start(
                c2w_sb[:].rearrange("p a b c -> p (a b c)"), c2w8[:, :])
            nc.sync.dma_start(
                W16[:], _ap(wre[:, :].tensor, 0,
                            [[CO, 128], [128 * CO, 16], [1, CO]]))
            wf3_sb = persist.tile([128, 2, 64, CO], F8)
            nc.sync.dma_start(
                wf3_sb[:].rearrange("p a b c -> p (a b c)"), wf3[:, :])

            # decoder weights (stream under the conv phase)
            dwsb = {}
            for nm, (kdim, ndim, win_dram) in (
                    ("1", (160, 512, d1)),
                    ("2", (512, 1024, d2)),
                    ("3", (1024, 1024, d3))):
                nkt = (kdim + 127) // 128
                wsb = dc.tile([128, nkt, ndim], F16, tag=f"w{nm}",
                              name=f"w{nm}")
                for kt in range(nkt):
                    ksz = min(128, kdim - kt * 128)
                    nc.sync.dma_start(
                        wsb[:ksz, kt, :],
                        _ap(win_dram[:, :].tensor, kt * 128 * ndim,
                            [[ndim, ksz], [1, ndim]]))
                dwsb[nm] = wsb
            d1b_sb = dc.tile([128, 4], F32)
            nc.sync.dma_start(d1b_sb[:], d1b[:, :])
            d2b_sb = dc.tile([128, 8], F32)
            nc.sync.dma_start(d2b_sb[:], d2b[:, :])
            d3br_sb = dc.tile([1, 1024], F16)
            nc.sync.dma_start(d3br_sb[:], d3br[:, :])

            # ====== conv1: r1c -> x1 fp8 [icb, ph, pw, h', w', b] ======
            with tc.tile_pool(name="c1psum", bufs=2, space="PSUM") as c1ps:
                NTOT = BL * 576  # 7488 per m
                for m in range(2):
                    off = 0
                    while off < NTOT:
                        csz = min(512, NTOT - off)
                        ps = c1ps.tile([128, 512], F32, tag="c1ps")
                        nc.tensor.matmul(ps[:, :csz],
                                         c1w_sb[0:81, m * 128:(m + 1) * 128],
                                         r1[0:81, off:off + csz])
                        xh = x1a[:, m].rearrange(
                            "p a c h w b -> p (a c h w b)")[:, off:off + csz]
                        nc.scalar.activation(xh, ps[:, :csz],
                                             ACT.Relu, bias=c1b_sb[:, m:m + 1])
                        off += csz

        # ============ conv2: fp8 DoubleRow, 4 chunk groups ============
        u_t = [persist.tile([128, BL, 8, 8], F32, tag=f"u_{m}",
                            name=f"u_{m}") for m in range(2)]
        hmax = sq.tile([128, 2, BL], F32)    # [c, m, b]
        hneg = sq.tile([128, 2, BL], F32)
        with tc.tile_pool(name="c2psum", bufs=2, space="PSUM") as c2ps:
            for m in range(2):
                for hc in range(2):
                    ps = c2ps.tile([128, 4, 104], F32, tag="c2ps")
                    for j in range(81):
                        kh, kw = divmod(j, 9)
                        ph, h0 = kh & 1, kh >> 1
                        pw, w0 = kw & 1, kw >> 1
                        rhs = _ap(x1a[:].tensor,
                                  x1a[:].offset + ph * 3744 + pw * 1872
                                  + (hc * 4 + h0) * 156 + w0 * 13,
                                  [list(x1a[:].ap[0]), [7488, 2],
                                   [156, 4], [1, 104]])
                        nc.tensor.matmul(ps[:], c2w_sb[:, :, j,
                                                       m * 128:(m + 1) * 128],
                                         rhs, start=(j == 0), stop=(j == 80),
                                         perf_mode=DR)
                    # readout: psum [p, oh(4), w(8), b(13)] -> u_t [p,b,oh,w]
                    pst = ps[:]
                    src = _ap(pst.tensor, pst.offset,
                              [list(pst.ap[0]), [1, BL], [104, 4], [13, 8]])
                    ut = u_t[m][:]
                    dst = _ap(ut.tensor, ut.offset + hc * 4 * 8,
                              [list(ut.ap[0]), [64, BL], [8, 4], [1, 8]])
                    nc.scalar.activation(dst, src, ACT.Identity,
                                         bias=c2b_sb[:, m:m + 1],
                                         scale=1.0 / (XS * WS))
                    # incremental squash maxima (hidden under next chunk)
                    xs = u_t[m][:, :, hc * 4:hc * 4 + 4, 0]  # [128, b, 4h]
                    red = sq.tile([128, BL], F32, tag="red")
                    nc.vector.tensor_reduce(red[:], xs, AX.X, OP.max)
                    if hc == 0:
                        nc.vector.tensor_copy(hmax[:, m, :], red[:])
                    else:
                        nc.vector.tensor_tensor(hmax[:, m, :], hmax[:, m, :],
                                                red[:], OP.max)
                    msk = sq.tile([128, BL, 4], I32, tag="msk")
                    nc.vector.tensor_single_scalar(msk[:], xs, 0.0, OP.is_lt)
                    xn = sq.tile([128, BL, 4], F32, tag="xn")
                    nc.vector.tensor_copy(
                        xn[:], negbig[:, 0:1].to_broadcast((128, BL, 4)))
                    nc.vector.copy_predicated(xn[:], msk[:], xs)
                    nc.vector.tensor_reduce(red[:], xn[:], AX.X, OP.max)
                    if hc == 0:
                        nc.vector.tensor_copy(hneg[:, m, :], red[:])
                    else:
                        nc.vector.tensor_tensor(hneg[:, m, :], hneg[:, m, :],
                                                red[:], OP.max)

        # ======== primary squash (value-threshold form) + u_sq ========
        # cross-partition max, replicated to all partitions
        redM = sq.tile([128, 2 * BL], F32)
        redN = sq.tile([128, 2 * BL], F32)
        hmax2 = hmax[:].rearrange("p m b -> p (m b)")
        hneg2 = hneg[:].rearrange("p m b -> p (m b)")
        nc.gpsimd.partition_all_reduce(redM[:], hmax2, channels=128,
                                       reduce_op=bass_isa.ReduceOp.max)
        nc.gpsimd.partition_all_reduce(redN[:], hneg2, channels=128,
                                       reduce_op=bass_isa.ReduceOp.max)
        Mb = sq.tile([128, BL], F32)
        Nb = sq.tile([128, BL], F32)
        nc.vector.tensor_tensor(Mb[:], redM[:, 0:BL],
                                redM[:, BL:2 * BL], OP.max)
        nc.vector.tensor_tensor(Nb[:], redN[:, 0:BL],
                                redN[:, BL:2 * BL], OP.max)

        usq_all = persist.tile([128, 2, BL, 8, 8], F8)
        for m in range(2):
            xs = u_t[m][:, :, :, 0]          # [128, b, h]
            y = sq.tile([128, BL, 8], F32, tag="y")
            aff = sq.tile([128, BL, 8], F32, tag="aff")
            mk = sq.tile([128, BL, 8], I32, tag="mk")
            mk2 = sq.tile([128, BL, 8], I32, tag="mk2")
            nc.vector.tensor_copy(y[:], xs)
            # x < mneg -> a2*x+b2
            nc.vector.tensor_tensor(
                mk[:], xs, Nb[:, :, None].to_broadcast((128, BL, 8)),
                OP.is_lt)
            nc.vector.tensor_scalar(aff[:], xs, a2, b2, OP.mult, OP.add)
            nc.vector.copy_predicated(y[:], mk[:], aff[:])
            # (x >= 0) & (x < M) -> a3*x+b3
            nc.vector.tensor_single_scalar(mk[:], xs, 0.0, OP.is_ge)
            nc.vector.tensor_tensor(
                mk2[:], xs, Mb[:, :, None].to_broadcast((128, BL, 8)),
                OP.is_lt)
            nc.vector.tensor_tensor(mk[:], mk[:], mk2[:], OP.mult)
            nc.vector.tensor_scalar(aff[:], xs, a3, b3, OP.mult, OP.add)
            nc.vector.copy_predicated(y[:], mk[:], aff[:])
            # u_sq_fp8 = 8 * y * u  (x8 for fp8 range; undone on receive)
            nc.vector.tensor_scalar(y[:], y[:], US, None, OP.mult)
            nc.vector.tensor_tensor(
                usq_all[:, m], u_t[m][:],
                y[:, :, :, None].to_broadcast((128, BL, 8, 8)), OP.mult)

        # scatter to send buffer [dest][b][(ch'=m*16+c)*64 + oh*8 + i]
        engs = [nc.sync, nc.scalar, nc.gpsimd]
        for m in range(2):
            for d in range(NCORES):
                dst = _ap(usq_send[:].tensor, d * (BL * RI) + m * 1024,
                          [[64, 16], [RI, BL], [1, 64]])
                engs[(m * NCORES + d) % 3].dma_start(
                    dst, usq_all[d * 16:(d + 1) * 16, m, :, :, :])

        # ============ AllToAll: u_sq -> route-sharded, full batch ========
        nc.gpsimd.collective_compute(
            "AllToAll", OP.bypass, replica_groups=GROUPS,
            ins=[usq_send[:]], outs=[usq_recv[:]])

        # ---- iter-0 s_j computed locally (hidden under the AllToAll):
        # c_ij is uniform (1/2048) so s_j(0) = mean over ALL routes of
        # u_hat, computable from this core's batch rows alone ----
        def digit_squash(rl, sjt, nr, sfx):
            x10 = _ap(sjt[:].tensor, sjt[:].offset,
                      [list(sjt[:].ap[0]), [16, 10]])
            cmp = rl.tile([nr, 10, 10], F32, tag="cmp" + sfx)
            x_j = _ap(sjt[:].tensor, sjt[:].offset,
                      [list(sjt[:].ap[0]), [16, 10], [0, 10]])
            x_k = _ap(sjt[:].tensor, sjt[:].offset,
                      [list(sjt[:].ap[0]), [0, 10], [16, 10]])
            nc.vector.tensor_tensor(cmp[:], x_j, x_k, OP.is_gt)
            r10 = rl.tile([nr, 10], F32, tag="r10" + sfx)
            nc.vector.tensor_reduce(r10[:], cmp[:], AX.X, OP.add)
            y = rl.tile([nr, 10], F32, tag="y" + sfx)
            tmp = rl.tile([nr, 10], F32, tag="tmp" + sfx)
            aff = rl.tile([nr, 10], F32, tag="aff" + sfx)
            mkA = rl.tile([nr, 10], I32, tag="mkA" + sfx)
            mkB = rl.tile([nr, 10], I32, tag="mkB" + sfx)
            cnt = rl.tile([nr, 4], F32, tag="cnt" + sfx)
            # i1
            nc.vector.tensor_single_scalar(tmp[:], x10, dt1, OP.is_lt)
            nc.vector.tensor_reduce(cnt[:, 0:1], tmp[:], AX.X, OP.add)
            # stage 1: r < i1 - 1
            nc.vector.tensor_copy(y[:], x10)
            nc.vector.tensor_scalar(tmp[:], cnt[:, 0:1].to_broadcast(
                (nr, 10)), 1.0, None, OP.subtract)
            nc.vector.tensor_tensor(mkA[:], r10[:], tmp[:], OP.is_lt)
            nc.vector.tensor_scalar(aff[:], x10, da1, db1, OP.mult, OP.add)
            nc.vector.copy_predicated(y[:], mkA[:], aff[:])
            # i2 on modified y
            nc.vector.tensor_single_scalar(tmp[:], y[:], 0.0, OP.is_lt)
            nc.vector.tensor_reduce(cnt[:, 1:2], tmp[:], AX.X, OP.add)
            # stage 2: (r >= i1) & (r < i2 - 1)
            nc.vector.tensor_tensor(
                mkA[:], r10[:], cnt[:, 0:1].to_broadcast((nr, 10)),
                OP.is_ge)
            nc.vector.tensor_scalar(tmp[:], cnt[:, 1:2].to_broadcast(
                (nr, 10)), 1.0, None, OP.subtract)
            nc.vector.tensor_tensor(mkB[:], r10[:], tmp[:], OP.is_lt)
            nc.vector.tensor_tensor(mkA[:], mkA[:], mkB[:], OP.mult)
            nc.vector.tensor_scalar(aff[:], y[:], da2, db2, OP.mult, OP.add)
            nc.vector.copy_predicated(y[:], mkA[:], aff[:])
            # i3 on modified y
            nc.vector.tensor_single_scalar(tmp[:], y[:], dt3, OP.is_lt)
            nc.vector.tensor_reduce(cnt[:, 2:3], tmp[:], AX.X, OP.add)
            # stage 3: (r >= i2) & (r < i3 - 1)
            nc.vector.tensor_tensor(
                mkA[:], r10[:], cnt[:, 1:2].to_broadcast((nr, 10)),
                OP.is_ge)
            nc.vector.tensor_scalar(tmp[:], cnt[:, 2:3].to_broadcast(
                (nr, 10)), 1.0, None, OP.subtract)
            nc.vector.tensor_tensor(mkB[:], r10[:], tmp[:], OP.is_lt)
            nc.vector.tensor_tensor(mkA[:], mkA[:], mkB[:], OP.mult)
            nc.vector.tensor_scalar(aff[:], y[:], da3, db3, OP.mult, OP.add)
            nc.vector.copy_predicated(y[:], mkA[:], aff[:])
            # stage 4: (r >= i3) & (r < 9)
            nc.vector.tensor_tensor(
                mkA[:], r10[:], cnt[:, 2:3].to_broadcast((nr, 10)),
                OP.is_ge)
            nc.vector.tensor_single_scalar(mkB[:], r10[:], 9.0, OP.is_lt)
            nc.vector.tensor_tensor(mkA[:], mkA[:], mkB[:], OP.mult)
            nc.vector.tensor_scalar(aff[:], y[:], da4, db4, OP.mult, OP.add)
            nc.vector.copy_predicated(y[:], mkA[:], aff[:])
            return y

        def make_v(rl, sjt, y, vdst, nr):
            # v_j = f * s_mod (s_mod[:, :, 0] = f)
            nc.vector.tensor_copy(vdst[:], sjt[:])
            vdst0 = _ap(vdst[:].tensor, vdst[:].offset,
                        [list(vdst[:].ap[0]), [16, 10]])
            nc.vector.tensor_copy(vdst0, y[:])
            f_b = _ap(y[:].tensor, y[:].offset,
                      [list(y[:].ap[0]), [1, 10], [0, 16]])
            nc.vector.tensor_tensor(
                vdst[:].rearrange("b (c o) -> b c o", c=10),
                vdst[:].rearrange("b (c o) -> b c o", c=10), f_b, OP.mult)

        with tc.tile_pool(name="s0", bufs=1) as s0pool, \
             tc.tile_pool(name="s0ps", bufs=1, space="PSUM") as s0ps:
            ps0 = s0ps.tile([BL, CO], F32)
            for ohi in range(64):
                nc.tensor.matmul(ps0[:],
                                 usq_all[:, :, :, ohi >> 3, ohi & 7],
                                 wf3_sb[:, :, ohi, :],
                                 start=(ohi == 0), stop=(ohi == 63),
                                 perf_mode=DR)
            sj0l = s0pool.tile([BL, CO], F32)
            nc.scalar.activation(sj0l[:], ps0[:], ACT.Identity,
                                 scale=1.0 / (US * WS2 * 2048.0))
            y0 = digit_squash(s0pool, sj0l, BL, "z")
            v0_16 = s0pool.tile([BL, CO], F16)
            v0 = s0pool.tile([BL, CO], F32)
            make_v(s0pool, sj0l, y0, v0, BL)
            nc.vector.tensor_copy(v0_16[:], v0[:])
            nc.sync.dma_start(
                _ap(ag_in[:].tensor, 0, [[CO, BL], [1, CO]]), v0_16[:])
            nc.gpsimd.collective_compute(
                "AllGather", OP.bypass, replica_groups=GROUPS,
                ins=[ag_in[:]], outs=[ag_out[:]])

        # ============ routing ============
        usq8_b = rt.tile([128, RI], F8)
        nc.sync.dma_start(
            usq8_b[0:BG, :], _ap(usq_recv[:].tensor, 0, [[RI, BG], [1, RI]]))
        with tc.tile_pool(name="tps", bufs=2, space="PSUM") as tps:
            # p-state warmers: keep the PE clocked up through the AllToAll
            psW = tps.tile([128, BG], F32, tag="psW")
            for dk in range(40):
                nc.tensor.matmul(psW[:], ident16[:, 0:128],
                                 ident16[:, 0:BG], start=True, stop=True)
            # un-scale the fp8 payload back to fp16 u_sq
            nc.vector.tensor_scalar(usq_b[0:BG, :], usq8_b[0:BG, :],
                                    1.0 / US, None, OP.mult)
            for t in range(16):
                pt = tps.tile([128, BG], F16, tag="pt")
                nc.tensor.transpose(pt[:], usq_b[0:BG, 128 * t:128 * (t + 1)],
                                    ident16[0:BG, 0:BG])
                nc.vector.tensor_copy(usq_T[:, t, :], pt[:])

        with tc.tile_pool(name="rloop", bufs=3) as rl, \
             tc.tile_pool(name="rpsS", bufs=1, space="PSUM") as rpsS, \
             tc.tile_pool(name="rpsT", bufs=1, space="PSUM") as rpsT, \
             tc.tile_pool(name="rps1", bufs=1, space="PSUM") as rps1:

            def agreement(v16):
                qall = rl.tile([128, 16, 10], F16, tag="qall")
                for half in range(2):
                    psT = rpsT.tile([128, 8, 256], F32, tag="psT")
                    for j in range(8):
                        t = half * 8 + j
                        nc.tensor.matmul(
                            psT[:, j, 0:CO],
                            usq_b[0:BG, 128 * t:128 * (t + 1)], v16[:])
                    T16 = rl.tile([128, 8, CO], F16, tag="T16")
                    nc.vector.tensor_copy(T16[:], psT[:, :, 0:CO])
                    prod = rl.tile([128, 8, CO], F16, tag="prod")
                    nc.vector.tensor_tensor(
                        prod[:], W16[:, 8 * half:8 * (half + 1), :],
                        T16[:], OP.mult)
                    with nc.allow_low_precision(
                            reason="16-term o-sum feeding small logits"):
                        nc.vector.tensor_reduce(
                            qall[:, 8 * half:8 * (half + 1), :],
                            prod[:].rearrange("p j (c o) -> p j c o", c=10),
                            AX.X, OP.add)
                psA = rpsS.tile([128, CO], F32, tag="psA")
                nc.tensor.matmul(psA[:], comb_sb[:],
                                 qall[:].rearrange("p t c -> p (t c)"))
                nc.vector.tensor_tensor(b_rep[:], b_rep[:], psA[:], OP.add)

            # iter 0: v_j(0) arrives via the AllGather
            v16_0 = rl.tile([BG, CO], F16, tag="v16", name="v16_0")
            nc.sync.dma_start(
                v16_0[:], _ap(ag_out[:].tensor, 0, [[CO, BG], [1, CO]]))
            agreement(v16_0)

            for it in (1, 2):
                if True:
                    cexp = rl.tile([128, CO], F16, tag="cexp")
                    nc.scalar.activation(cexp[:], b_rep[:], ACT.Exp)
                    mc = rl.tile([128, 16, CO], F16, tag="mc")
                    cexp_b = _ap(cexp[:].tensor, cexp[:].offset,
                                 [list(cexp[:].ap[0]), [10, 16], [1, 10],
                                  [0, 16]])
                    nc.vector.tensor_tensor(
                        mc[:].rearrange("p t (c o) -> p t c o", c=10),
                        W16[:].rearrange("p t (c o) -> p t c o", c=10),
                        cexp_b, OP.mult)
                    # E_c partial
                    psE = rps1.tile([1, CO], F32, tag="psE")
                    nc.tensor.matmul(psE[:], ones8[:], cexp[:])
                    E10 = rl.tile([1, 10], F16, tag="E10")
                    psE_v = _ap(psE[:].tensor, psE[:].offset,
                                [list(psE[:].ap[0]), [1, 10], [10, 16]])
                    with nc.allow_low_precision(
                            reason="E sums ~2048, fp16 ok (validated)"):
                        nc.vector.tensor_reduce(E10[:], psE_v, AX.X, OP.add)
                # s_tilde
                psS = rpsS.tile([BG, CO], F32, tag="psS")
                for t in range(16):
                    nc.tensor.matmul(psS[:], usq_T[:, t, :], mc[:, t, :],
                                     start=(t == 0), stop=(t == 15))
                s_sb = rl.tile([BG, CO], F16, tag="s_sb")
                nc.vector.tensor_copy(s_sb[:], psS[:])
                nc.sync.dma_start(
                    _ap(cc_in[it][:].tensor, 0, [[CO, BG], [1, CO]]), s_sb[:])
                if it > 0:
                    nc.sync.dma_start(
                        _ap(cc_in[it][:].tensor, BG * CO, [[1, 1], [1, 10]]),
                        E10[:])
                # p-state warmers: keep the PE clocked up through the
                # collective so post-AR matmuls run at full rate
                for dk in range(36):
                    nc.tensor.matmul(psS[:, 0:BG], usq_T[:, 0, :],
                                     usq_T[:, 1, :], start=True, stop=True)
                nc.gpsimd.collective_compute(
                    "AllReduce", OP.add, replica_groups=GROUPS,
                    ins=[cc_in[it][:]], outs=[cc_out[it][:]])
                s_full = rl.tile([BG, CO], F16, tag="s_full")
                nc.sync.dma_start(
                    s_full[:],
                    _ap(cc_out[it][:].tensor, 0, [[CO, BG], [1, CO]]))
                sj = rl.tile([BG, CO], F32, tag="sj")
                sf32 = rl.tile([BG, CO], F32, tag="sf32")
                nc.vector.tensor_copy(sf32[:], s_full[:])
                E10r = rl.tile([1, 10], F16, tag="E10r")
                nc.sync.dma_start(
                    E10r[:],
                    _ap(cc_out[it][:].tensor, BG * CO, [[1, 1], [1, 10]]))
                E32 = rl.tile([1, 10], F32, tag="E32")
                nc.vector.tensor_copy(E32[:], E10r[:])
                rE = rl.tile([1, 10], F32, tag="rE")
                nc.vector.reciprocal(rE[:], E32[:])
                psBE = rps1.tile([BG, CO], F32, tag="psBE")
                rE_b = _ap(rE[:].tensor, rE[:].offset,
                           [list(rE[:].ap[0]), [1, 10], [0, 16]])
                nc.tensor.matmul(psBE[:], ones_r104[:], rE_b)
                nc.vector.tensor_tensor(sj[:], sf32[:], psBE[:],
                                        OP.mult)

                # ---- digit squash (exact rank arithmetic) ----
                y = digit_squash(rl, sj, BG, "")
                if it == 2:
                    vdst = vj
                else:
                    vdst = rl.tile([BG, CO], F32, tag="vtmp", name="vtmp")
                make_v(rl, sj, y, vdst, BG)

                if it == 1:
                    v16 = rl.tile([BG, CO], F16, tag="v16", name="v16")
                    nc.vector.tensor_copy(v16[:], vdst[:])
                    agreement(v16)

        # ============ decoder (all 104 rows, identical on every core) ====
        with tc.tile_pool(name="dps", bufs=2, space="PSUM") as dps:
            sqv = dc.tile([BG, CO], F32)
            nc.scalar.activation(sqv[:], vj[:], ACT.Square)
            csum = dc.tile([BG, 10], F32)
            sq_v = _ap(sqv[:].tensor, sqv[:].offset,
                       [list(sqv[:].ap[0]), [16, 10], [1, 16]])
            nc.vector.tensor_reduce(csum[:], sq_v, AX.X, OP.add)
            classes = dc.tile([BG, 10], F32)
            nc.scalar.activation(classes[:], csum[:], ACT.Sqrt)
            expcl = dc.tile([BG, 10], F32)
            nc.scalar.activation(expcl[:], classes[:], ACT.Exp)
            nc.vector.tensor_scalar_mul(expcl[:], expcl[:], bmask_sb[:, 0:1])
            psD = dps.tile([10, 1], F32, tag="dsmall")
            nc.tensor.matmul(psD[:], expcl[:], ones104[:])
            dsb = dc.tile([10, 1], F32)
            nc.vector.tensor_copy(dsb[:], psD[:])
            psDT = dps.tile([1, 10], F32, tag="dsmall")
            nc.tensor.transpose(psDT[:], dsb[:], ident[0:10, 0:10])
            dT = dc.tile([1, 10], F32)
            nc.vector.tensor_copy(dT[:], psDT[:])
            rD = dc.tile([1, 10], F32)
            nc.vector.reciprocal(rD[:], dT[:])
            psBD = dps.tile([BG, 10], F32, tag="dsmall")
            rD_b = _ap(rD[:].tensor, rD[:].offset,
                       [list(rD[:].ap[0]), [1, 10]])
            nc.tensor.matmul(psBD[:], ones_r104[:], rD_b)
            p = dc.tile([BG, 10], F32)
            nc.vector.tensor_tensor(p[:], expcl[:], psBD[:], OP.mult)
            pm = dc.tile([BG, 1], F32)
            nc.vector.tensor_reduce(pm[:], p[:], AX.X, OP.max)
            mask = dc.tile([BG, 10], F32)
            nc.vector.tensor_tensor(mask[:], p[:],
                                    pm[:].to_broadcast((BG, 10)), OP.is_ge)
            tm = dc.tile([BG, CO], F16)
            mask_b = _ap(mask[:].tensor, mask[:].offset,
                         [list(mask[:].ap[0]), [1, 10], [0, 16]])
            nc.vector.tensor_tensor(
                tm[:].rearrange("b (c o) -> b c o", c=10),
                vj[:].rearrange("b (c o) -> b c o", c=10), mask_b, OP.mult)
            nc.sync.dma_start(out[:, 0:160], vj[:])

            # tT [160, 104] via PE transposes
            tT = dc.tile([128, 2, BG], F16)
            for kt in range(2):
                ksz = 128 if kt == 0 else 32
                pst = dps.tile([128, BG], F16, tag="dpst")
                nc.tensor.transpose(pst[:ksz, :],
                                    tm[:, kt * 128:kt * 128 + ksz],
                                    ident16[0:BG, 0:BG])
                nc.vector.tensor_copy(tT[:ksz, kt, :], pst[:ksz, :])

            # L1/L2 weight-stationary -> transposed activations
            h1T = dc.tile([128, 4, BG], F16)
            for mt in range(4):
                psH = dps.tile([128, BG], F32, tag="dpsH")
                for kt in range(2):
                    ksz = 128 if kt == 0 else 32
                    nc.tensor.matmul(
                        psH[:], dwsb["1"][:ksz, kt, mt * 128:(mt + 1) * 128],
                        tT[:ksz, kt, :], start=(kt == 0), stop=(kt == 1))
                nc.scalar.activation(h1T[:, mt, :], psH[:], ACT.Relu,
                                     bias=d1b_sb[:, mt:mt + 1])
            h2T = dc.tile([128, 8, BG], F16)
            for mt in range(8):
                psH = dps.tile([128, BG], F32, tag="dpsH")
                for kt in range(4):
                    nc.tensor.matmul(
                        psH[:], dwsb["2"][:, kt, mt * 128:(mt + 1) * 128],
                        h1T[:, kt, :], start=(kt == 0), stop=(kt == 3))
                nc.scalar.activation(h2T[:, mt, :], psH[:], ACT.Relu,
                                     bias=d2b_sb[:, mt:mt + 1])
            # L3 moving-form: r3 [104, 1024] batch-major, bias via K=1 row
            r3 = dc.tile([BG, 1024], F32)
            for half in range(2):
                psR = dps.tile([BG, 512], F32, tag="dpsR")
                for kt in range(8):
                    nc.tensor.matmul(
                        psR[:], h2T[:, kt, :],
                        dwsb["3"][:, kt, half * 512:(half + 1) * 512],
                        start=(kt == 0), stop=False)
                nc.tensor.matmul(
                    psR[:], ones1_16[:],
                    d3br_sb[0:1, half * 512:(half + 1) * 512],
                    start=False, stop=True)
                nc.scalar.activation(r3[:, half * 512:(half + 1) * 512],
                                     psR[:], ACT.Sigmoid)
            nc.sync.dma_start(out[:, 160:1184], r3[:])

        dc.release()
        rt.release()
        sq.release()
        persist.release()
        const.release()

    nc.compile()
    return nc


_PROGRAM = None


def _get_program():
    global _PROGRAM
    if _PROGRAM is None:
        _PROGRAM = build_program()
    return _PROGRAM


def _prepare_in_maps(inputs):
    data = np.asarray(inputs["data"], dtype=np.float32)      # (100,1,32,32)
    conv1_w = np.asarray(inputs["conv1_w"], dtype=np.float32)
    conv1_b = np.asarray(inputs["conv1_b"], dtype=np.float32)
    prim_w = np.asarray(inputs["prim_w"], dtype=np.float32)
    prim_b = np.asarray(inputs["prim_b"], dtype=np.float32)
    W_dc = np.asarray(inputs["W_dc"], dtype=np.float32)
    dec_w1 = np.asarray(inputs["dec_w1"], dtype=np.float32)
    dec_b1 = np.asarray(inputs["dec_b1"], dtype=np.float32)
    dec_w2 = np.asarray(inputs["dec_w2"], dtype=np.float32)
    dec_b2 = np.asarray(inputs["dec_b2"], dtype=np.float32)
    dec_w3 = np.asarray(inputs["dec_w3"], dtype=np.float32)
    dec_b3 = np.asarray(inputs["dec_b3"], dtype=np.float32)

    B = data.shape[0]
    data_pad = np.zeros((BG, 32, 32), np.float32)
    data_pad[:B] = data[:, 0]
    swv = np.lib.stride_tricks.sliding_window_view(data_pad, (24, 24),
                                                   axis=(1, 2))
    # swv[b, kh, kw, oh, ow] = data[b, oh+kh, ow+kw]
    # columns (ph=oh&1, pw=ow&1, h'=oh>>1, w'=ow>>1, b)
    t5 = swv.transpose(1, 2, 0, 3, 4).reshape(81, BG, 12, 2, 12, 2)
    r1c_all = np.ascontiguousarray(
        t5.transpose(0, 3, 5, 2, 4, 1)).astype(np.float16)  # [81,ph,pw,h,w,b]

    c1w = np.ascontiguousarray(
        conv1_w.transpose(2, 3, 1, 0).reshape(81, 256)).astype(np.float16)
    c1w = (c1w.astype(np.float32) * XS).astype(np.float16)
    c1b = np.zeros((128, 2), np.float32)
    c1b[:, 0] = conv1_b[:128] * XS
    c1b[:, 1] = conv1_b[128:] * XS
    # conv2 weights: [p, icb, tap, oc] * WS -> fp8
    c2w8 = np.ascontiguousarray(
        prim_w.transpose(1, 2, 3, 0).reshape(2, 128, 81, 256)
        .transpose(1, 0, 2, 3)).reshape(128, 2 * 81 * 256)
    c2w8 = (c2w8 * WS).astype(ml_dtypes.float8_e4m3fn)
    c2b = np.zeros((128, 2), np.float32)
    c2b[:, 0] = prim_b[:128]
    c2b[:, 1] = prim_b[128:]
    comb = np.zeros((128, 128), np.float16)
    for blk in range(16):
        comb[blk * 8:(blk + 1) * 8, blk * 8:(blk + 1) * 8] = 0.01
    # full W_dc for the local iter-0 s_j: [p, mh, (oh,i), (c,o)] * WS2, fp8
    wf3 = np.ascontiguousarray(
        W_dc.reshape(2, 128, 8, 10, 16, 8).transpose(1, 0, 2, 5, 3, 4)
        .reshape(128, 2 * 64 * CO) * WS2)
    wf3 = np.clip(wf3, -240, 240).astype(ml_dtypes.float8_e4m3fn)
    d1 = np.ascontiguousarray(dec_w1.T).astype(np.float16)
    d1b = np.ascontiguousarray(dec_b1.reshape(4, 128).T)
    d2 = np.ascontiguousarray(dec_w2.T).astype(np.float16)
    d2b = np.ascontiguousarray(dec_b2.reshape(8, 128).T)
    d3 = np.ascontiguousarray(dec_w3.T).astype(np.float16)
    d3br = np.ascontiguousarray(dec_b3.reshape(1, 1024)).astype(np.float16)
    bm = np.zeros((BG, 1), np.float32)
    bm[:B] = 1.0

    # route shard: core k, r' = (ch', oh), ch' = m*16 + c_loc,
    # global ch = m*128 + k*16 + c_loc
    rp = np.arange(256)
    chp = rp >> 3
    oh = rp & 7
    m_ = chp >> 4
    c_loc = chp & 15

    in_maps = []
    for k in range(NCORES):
        gch = m_ * 128 + k * 16 + c_loc
        gr = gch * 8 + oh                       # global route index
        # wre rows (r', i): [256, 8, 160] from W_dc[gr] [10, 16, 8]
        wk = W_dc[gr]                           # [256, 10, 16, 8]
        wre = np.ascontiguousarray(
            wk.transpose(0, 3, 1, 2).reshape(RI, CO)).astype(np.float16)
        in_maps.append({
            "r1c": np.ascontiguousarray(
                r1c_all[:, :, :, :, :, k * BL:(k + 1) * BL]
                .reshape(81, BL * 576)),
            "c1w": c1w, "c1b": c1b, "c2w8": c2w8, "c2b": c2b,
            "wre": wre, "wf3": wf3, "comb": comb, "bmask": bm,
            "d1": d1, "d1b": d1b, "d2": d2, "d2b": d2b,
            "d3": d3, "d3br": d3br,
        })

    return in_maps, B


def kernel(**inputs):
    in_maps, B = _prepare_in_maps(inputs)
    nc = _get_program()
    res = run_bass_kernel_spmd(nc, in_maps, list(range(NCORES)))
    return res.results[0]["out"][:B]


def timed_run(inputs):
    in_maps, _ = _prepare_in_maps(inputs)
    nc = _get_program()
    res = run_bass_kernel_spmd(nc, in_maps, list(range(NCORES)), trace=True)
    if res.exec_time_ns is None:
        raise RuntimeError("exec_time_ns unavailable")
    return res.exec_time_ns
